# revision 1
# baseline (speedup 1.0000x reference)
"""Trainium2 Bass kernel for nn_EncoderImage (gnn_message_passing).

Strategy: pure data-parallel over batch (32 images/core x 8 cores).
All MLP math runs feature-major ([feature_partitions, row_free]) so weight
tiles DMA directly as matmul lhsT and biases are per-partition ACT operands.

The topk/gather message passing is rewritten exactly (img_range values are
only 0/1, and jax.lax.top_k tie-breaks by lowest index):
    out[b] = (W0[b] + diag(relu(5 - c[b]))) @ (m[b] * v[b])
where W0[b][k,j] = r[k,j] * (cumsum_j r[k,:] <= 5) and c[b][k] = sum_j r[k,j].
The cumsum is a matmul against a constant triangular matrix; per 3-image
group the 36x36 W' blocks are assembled block-diagonally by a selector
matmul + block mask, so the whole group is one K=108 stationary operand.

Matmuls are bf16 (inputs rounded; fp32 PSUM accumulation); norms/biases fp32.

Hardware constraints honored: engine APs start at partition 0/32/64/96;
at most one PSUM input per DVE op; no tensor_tensor_reduce.
"""

import numpy as np
import ml_dtypes

import concourse.bacc as bacc
import concourse.bass as bass
import concourse.tile as tile
import concourse.mybir as mybir
from concourse.bass_utils import run_bass_kernel_spmd
from concourse.masks import make_identity

F32 = mybir.dt.float32
BF16 = mybir.dt.bfloat16
FP8 = mybir.dt.float8e4
AF = mybir.ActivationFunctionType
OP = mybir.AluOpType
NPBF16 = ml_dtypes.bfloat16
NPFP8 = ml_dtypes.float8_e4m3
WSCALE = 16.0

B, K, D, E = 256, 36, 2048, 1024
NCORES = 8
BSH = B // NCORES           # 32 images per core
R = BSH * K                 # 1152 rows per core
NT, NW = 3, 384             # row windows per psum accumulation group
KC1 = 18                    # k-chunks for x (2048 img + 4 bbox + 1 area)
KC = 16                     # k-chunks for D
MC = 16                     # m-chunks for D outputs
ECH = 8                     # m-chunks for E outputs
GROUPS = [(i, 3) for i in range(0, 30, 3)] + [(30, 2)]   # (start_img, n_imgs)


# ---------------------------------------------------------------- program ---

def _declare(nc):
    t = {}
    def inp(name, shape, dt):
        t[name] = nc.dram_tensor(name, list(shape), dt, kind="ExternalInput").ap()
    inp("imT", (128, KC, R), FP8)
    inp("im", (R, D), BF16)
    inp("bbT", (4, R), BF16)
    inp("rT", (128, R), BF16)
    inp("L36", (128, K), BF16)
    inp("sel", (4, 2), BF16)
    inp("Sb", (128, 108), BF16)
    inp("bmask", (108, 108), BF16)
    inp("gw1", (MC, 128, KC1, 128), FP8)
    inp("nw1", (MC, 128, KC1, 128), FP8)
    inp("nw2", (MC, 128, KC, 128), FP8)
    inp("mw1", (MC, 128, KC, 128), BF16)
    inp("mw2", (ECH, 128, KC, 128), BF16)
    inp("gw2", (128, MC), FP8)
    inp("gb1", (128, MC), F32)
    inp("nb1", (128, MC), F32)
    inp("nb2", (128, MC), F32)
    inp("mb1", (128, MC), F32)
    inp("mb2", (128, ECH), F32)
    inp("gb2", (1, 1), F32)
    t["outp"] = nc.dram_tensor("outp", [R, E], F32, kind="ExternalOutput").ap()
    return t


def _emit(nc, tc, T):
    P = 128

    # ----- whole-kernel pools (left side) -----
    const = tc.alloc_tile_pool(name="const", bufs=1, side="left")
    wpool = tc.alloc_tile_pool(name="wts", bufs=2, side="left")
    misc = tc.alloc_tile_pool(name="misc", bufs=1, side="left")

    identb = const.tile([P, P], BF16, name="identb", tag="identb")
    make_identity(nc, identb)
    identf = const.tile([P, P], F32, name="identf", tag="identf")
    make_identity(nc, identf)
    ones_b = const.tile([P, 1], BF16, name="ones_b", tag="ones_b")
    nc.vector.memset(ones_b[:], 1.0)
    five_t = const.tile([1, 1], F32, name="five_t", tag="five_t")
    nc.vector.memset(five_t[:], 5.0)
    L36t = const.tile([P, K], BF16, name="L36t", tag="L36t")
    nc.sync.dma_start(L36t[:], T["L36"][:])
    selt = const.tile([4, 2], BF16, name="selt", tag="selt")
    nc.sync.dma_start(selt[:], T["sel"][:])
    Sbt = const.tile([P, 108], BF16, name="Sbt", tag="Sbt")
    nc.sync.dma_start(Sbt[:], T["Sb"][:])
    bmaskt = const.tile([108, 108], BF16, name="bmaskt", tag="bmaskt")
    nc.sync.dma_start(bmaskt[:], T["bmask"][:])
    rTt = const.tile([P, R], BF16, name="rTt", tag="rTt")
    nc.sync.dma_start(rTt[:], T["rT"][:])
    gw2t = const.tile([P, MC], FP8, name="gw2t", tag="gw2t")
    nc.sync.dma_start(gw2t[:], T["gw2"][:])
    btiles = {}
    for bn, cols in (("gb1", MC), ("nb1", MC), ("nb2", MC), ("mb1", MC), ("mb2", ECH)):
        bt = const.tile([P, cols], F32, name=bn + "t", tag=bn + "t")
        nc.sync.dma_start(bt[:], T[bn][:])
        btiles[bn] = bt
    gb2t = const.tile([1, 1], F32, name="gb2t", tag="gb2t")
    nc.sync.dma_start(gb2t[:], T["gb2"][:])
    bbT = misc.tile([4, R], BF16, name="bbT", tag="bbT")
    nc.sync.dma_start(bbT[:], T["bbT"][:])

    # ----- xT build: one [128, 18, R] fp8 tile -----
    pd_pool = tc.alloc_tile_pool(name="pd", bufs=2, space="PSUM")
    xpool = tc.alloc_tile_pool(name="xT", bufs=1, side="left")
    xTall = xpool.tile([P, KC1, R], FP8, name="xTall", tag="xTall")
    nc.sync.dma_start(xTall[:, 0:KC, :], T["imT"][:])
    nc.vector.memset(xTall[:, KC:KC1, :], 0.0)
    nc.vector.tensor_scalar_mul(xTall[0:4, KC, :], bbT[0:4, :], 0.1)
    d1s = misc.tile([1, R], F32, name="d1s", tag="d1s")
    for n in range(NT):
        sl = slice(n * NW, (n + 1) * NW)
        pd1 = pd_pool.tile([1, NW], F32, name=f"pd1_{n}", tag="pd1")
        nc.tensor.matmul(pd1[:], selt[:, 0:1], bbT[:, sl], start=True, stop=True)
        pd2 = pd_pool.tile([1, NW], F32, name=f"pd2_{n}", tag="pd2")
        nc.tensor.matmul(pd2[:], selt[:, 1:2], bbT[:, sl], start=True, stop=True)
        nc.scalar.copy(d1s[0:1, sl], pd1[:])
        # area*0.1 = (d2 * 0.1) * d1
        nc.vector.scalar_tensor_tensor(xTall[0:1, KC1 - 1, sl], pd2[:], 0.1,
                                       d1s[0:1, sl], OP.mult, OP.mult)
    pd_pool.release()

    # ----- MLP layer helpers (feature-major) -----
    DR = mybir.MatmulPerfMode.DoubleRow

    def mlp(lname, wap, nkc, bias_tile, func, rhs, out_pool, mcs, pspool,
            out_dtype=BF16):
        """bf16 layer: rhs is a list of [128, R] tiles."""
        outs = []
        for mc in range(mcs):
            wt = wpool.tile([P, nkc, P], BF16, name=f"w_{lname}_{mc}", tag="wstream",
                            padded_shape=[P, KC1, P])
            nc.sync.dma_start(wt[:], wap[mc])
            ot = out_pool.tile([P, R], out_dtype, name=f"{lname}_{mc}",
                               tag=f"{lname}_{mc}")
            for n in range(NT):
                ps = pspool.tile([P, NW], F32, name=f"ps_{lname}_{mc}_{n}", tag="pmm")
                for kc in range(nkc):
                    nc.tensor.matmul(ps[:], wt[:, kc, :],
                                     rhs[kc][:, n * NW:(n + 1) * NW],
                                     start=(kc == 0), stop=(kc == nkc - 1))
                nc.scalar.activation(ot[:, n * NW:(n + 1) * NW], ps[:], func,
                                     bias=bias_tile[:, mc:mc + 1])
            outs.append(ot)
        return outs

    def mlp8(lname, wap, nkc, bias_tile, func, rhs_all, mcs, pspool,
             out_all=None, out_pool=None, out_dtype=BF16):
        """fp8 DoubleRow layer: rhs_all is one [128, nkc, R] fp8 tile;
        weights are pre-scaled by WSCALE, descaled in the ACT eviction.
        Writes into big tile out_all [128, mcs, R] if given, else returns
        a list of [128, R] tiles from out_pool."""
        outs = []
        for mc in range(mcs):
            wt = wpool.tile([P, nkc, P], FP8, name=f"w_{lname}_{mc}", tag="wstream8",
                            padded_shape=[P, KC1, P])
            nc.sync.dma_start(wt[:], wap[mc])
            if out_all is None:
                ot = out_pool.tile([P, R], out_dtype, name=f"{lname}_{mc}",
                                   tag=f"{lname}_{mc}")
                outs.append(ot)
            for n in range(NT):
                nsl = slice(n * NW, (n + 1) * NW)
                ps = pspool.tile([P, NW], F32, name=f"ps_{lname}_{mc}_{n}", tag="pmm")
                for kc in range(0, nkc, 2):
                    nc.tensor.matmul(ps[:], wt[:, kc:kc + 2, :],
                                     rhs_all[:, kc:kc + 2, nsl],
                                     start=(kc == 0), stop=(kc == nkc - 2),
                                     perf_mode=DR)
                dst = out_all[:, mc, nsl] if out_all is not None else ot[:, nsl]
                nc.scalar.activation(dst, ps[:], func, scale=1.0 / WSCALE,
                                     bias=bias_tile[:, mc:mc + 1])
        return outs

    # ----- gate MLP -----
    pmm = tc.alloc_tile_pool(name="pmm", bufs=6, space="PSUM")
    psg_pool = tc.alloc_tile_pool(name="psg", bufs=2, space="PSUM")
    hg_pool = tc.alloc_tile_pool(name="hg", bufs=1, side="left")
    hgall = hg_pool.tile([P, MC, R], FP8, name="hgall", tag="hgall")
    mlp8("hg", T["gw1"], KC1, btiles["gb1"], AF.Relu, xTall, MC, pmm,
         out_all=hgall)

    m_row = misc.tile([32, R], F32, name="m_row", tag="m_row")
    nc.vector.memset(m_row[:], 0.0)
    for n in range(NT):
        psg = psg_pool.tile([1, NW], F32, name=f"psg{n}", tag="psg")
        for kc in range(MC):
            nc.tensor.matmul(psg[:], gw2t[:, kc:kc + 1],
                             hgall[:, kc, n * NW:(n + 1) * NW],
                             start=(kc == 0), stop=(kc == MC - 1))
        nc.scalar.activation(m_row[0:1, n * NW:(n + 1) * NW], psg[:], AF.Sigmoid,
                             scale=1.0 / WSCALE, bias=gb2t[0:1, 0:1])
    psg_pool.release()
    hg_pool.release()

    # ----- W0 build (cumsum mask) + w = relu(5 - c) -----
    psc_pool = tc.alloc_tile_pool(name="psc", bufs=1, space="PSUM")
    w0a = misc.tile([P, R], BF16, name="w0a", tag="w0a")
    nc.vector.memset(w0a[:], 0.0)
    w_row = misc.tile([32, R], F32, name="w_row", tag="w_row")
    nc.vector.memset(w_row[:], 0.0)
    for n in range(NT):
        sl = slice(n * NW, (n + 1) * NW)
        psc = psc_pool.tile([K, NW], F32, name=f"psc{n}", tag="psc")
        nc.tensor.matmul(psc[:], L36t[:], rTt[:, sl], start=True, stop=True)
        nc.vector.scalar_tensor_tensor(w0a[0:K, sl], psc[:], 5.0, rTt[0:K, sl],
                                       OP.is_le, OP.mult)
        pcc = psc_pool.tile([1, NW], F32, name=f"pcc{n}", tag="pcc")
        nc.tensor.matmul(pcc[:], ones_b[:], rTt[:, sl], start=True, stop=True)
        nc.scalar.activation(w_row[0:1, sl], pcc[:], AF.Relu,
                             bias=five_t[0:1, 0:1], scale=-1.0)
    psc_pool.release()

    # ----- node MLP -----
    hn_pool = tc.alloc_tile_pool(name="hn", bufs=1, side="right")
    hnall = hn_pool.tile([P, MC, R], FP8, name="hnall", tag="hnall")
    mlp8("hn", T["nw1"], KC1, btiles["nb1"], AF.Relu, xTall, MC, pmm,
         out_all=hnall)
    xpool.release()
    v_pool = tc.alloc_tile_pool(name="v", bufs=1, side="left")
    v = mlp8("v", T["nw2"], KC, btiles["nb2"], AF.Identity, hnall, MC, pmm,
             out_pool=v_pool)
    hn_pool.release()
    pmm.release()

    # ----- message passing, per image group -----
    imgsT_pool = tc.alloc_tile_pool(name="imgsT", bufs=1, side="right")
    gpool = tc.alloc_tile_pool(name="gp", bufs=2, side="right")
    ptrv_pool = tc.alloc_tile_pool(name="ptrv", bufs=3, space="PSUM")
    ptr2_pool = tc.alloc_tile_pool(name="ptr2", bufs=2, space="PSUM")
    pmsg_pool = tc.alloc_tile_pool(name="pmsg", bufs=2, space="PSUM")
    pg2_pool = tc.alloc_tile_pool(name="pg2", bufs=1, space="PSUM")
    imgsT = []
    for dc in range(MC):
        it = imgsT_pool.tile([P, R], BF16, name=f"imgsT{dc}", tag=f"imgsT{dc}")
        imgsT.append(it)

    for g, (i0, ng) in enumerate(GROUPS):
        rows = ng * K
        c0 = i0 * K

        # per-row scalars: mT (sigmoid gate) and wT (self-fallback count)
        pmw1 = ptrv_pool.tile([108, 32], F32, name=f"pmw1_{g}", tag="ptrv")
        nc.tensor.transpose(pmw1[0:rows, :], m_row[:, c0:c0 + rows],
                            identf[0:32, 0:32])
        pmw2 = ptrv_pool.tile([108, 32], F32, name=f"pmw2_{g}", tag="ptrv")
        nc.tensor.transpose(pmw2[0:rows, :], w_row[:, c0:c0 + rows],
                            identf[0:32, 0:32])
        mwT = gpool.tile([108, 2], F32, name=f"mwT{g}", tag="mwT")
        nc.vector.tensor_copy(mwT[0:rows, 0:1], pmw1[0:rows, 0:1])
        nc.vector.tensor_copy(mwT[0:rows, 1:2], pmw2[0:rows, 0:1])

        # block-diagonal stationary operand gW = blockdiag(W0^T) + diag(wT)
        pg2 = pg2_pool.tile([108, 108], F32, name=f"pg2_{g}", tag="pg2")
        nc.tensor.matmul(pg2[0:rows, 0:rows], Sbt[:, 0:rows],
                         w0a[:, c0:c0 + rows], start=True, stop=True)
        gW = gpool.tile([108, 108], BF16, name=f"gW{g}", tag="gW")
        nc.vector.tensor_mul(gW[0:rows, 0:rows], pg2[0:rows, 0:rows],
                             bmaskt[0:rows, 0:rows])
        tmpd = gpool.tile([108, 108], BF16, name=f"tmpd{g}", tag="tmpd")
        nc.vector.tensor_scalar_mul(tmpd[0:rows, 0:rows], identb[0:rows, 0:rows],
                                    mwT[0:rows, 1:2])
        nc.vector.tensor_add(gW[0:rows, 0:rows], gW[0:rows, 0:rows],
                             tmpd[0:rows, 0:rows])

        # u = m * v, transposed to row-major [rows, D]
        u = gpool.tile([108, D], BF16, name=f"u{g}", tag="u")
        for dc in range(MC):
            psv = ptrv_pool.tile([108, P], BF16, name=f"psv{g}_{dc}", tag="ptrv")
            nc.tensor.transpose(psv[0:rows, :], v[dc][:, c0:c0 + rows], identb[:])
            nc.vector.tensor_scalar_mul(u[0:rows, dc * P:(dc + 1) * P],
                                        psv[0:rows, :], mwT[0:rows, 0:1])

        # out = gW^T @ u  [rows, D], evicted to SBUF via ACT
        out_sb = gpool.tile([108, D], F32, name=f"outsb{g}", tag="outsb")
        for q in range(4):
            sl = slice(q * 512, (q + 1) * 512)
            psm = pmsg_pool.tile([108, 512], F32, name=f"psm{g}_{q}", tag="pmsg")
            nc.tensor.matmul(psm[0:rows, :], gW[0:rows, 0:rows], u[0:rows, sl],
                             start=True, stop=True)
            nc.scalar.copy(out_sb[0:rows, sl], psm[0:rows, :])

        # l2norm + residual add with original images
        sq = gpool.tile([108, 512], F32, name=f"sq{g}", tag="sq")
        nsq = gpool.tile([108, 8], F32, name=f"nsq{g}", tag="nsq")
        for q in range(4):
            sl = slice(q * 512, (q + 1) * 512)
            nc.vector.tensor_mul(sq[0:rows, :], out_sb[0:rows, sl],
                                 out_sb[0:rows, sl])
            nc.vector.tensor_reduce(nsq[0:rows, q:q + 1], sq[0:rows, :],
                                    axis=mybir.AxisListType.X, op=OP.add)
        nc.vector.tensor_add(nsq[0:rows, 4:5], nsq[0:rows, 0:1], nsq[0:rows, 1:2])
        nc.vector.tensor_add(nsq[0:rows, 5:6], nsq[0:rows, 2:3], nsq[0:rows, 3:4])
        nc.vector.tensor_add(nsq[0:rows, 6:7], nsq[0:rows, 4:5], nsq[0:rows, 5:6])
        nc.scalar.sqrt(nsq[0:rows, 7:8], nsq[0:rows, 6:7])
        inv = gpool.tile([108, 2], F32, name=f"inv{g}", tag="inv")
        nc.vector.tensor_scalar_add(inv[0:rows, 0:1], nsq[0:rows, 7:8], 1e-8)
        nc.vector.reciprocal(inv[0:rows, 1:2], inv[0:rows, 0:1])

        img_ld = gpool.tile([108, D], BF16, name=f"img{g}", tag="img")
        nc.sync.dma_start(img_ld[0:rows, :], T["im"][c0:c0 + rows, :])
        imgsw = gpool.tile([108, D], BF16, name=f"imgsw{g}", tag="imgsw")
        for q in range(4):
            sl = slice(q * 512, (q + 1) * 512)
            nc.vector.scalar_tensor_tensor(imgsw[0:rows, sl], out_sb[0:rows, sl],
                                           inv[0:rows, 1:2], img_ld[0:rows, sl],
                                           OP.mult, OP.add)

        # transpose back to feature-major imgsT
        for dc in range(MC):
            pst2 = ptr2_pool.tile([P, 108], BF16, name=f"pst2_{g}_{dc}", tag="ptr2")
            nc.tensor.transpose(pst2[:, 0:rows], imgsw[0:rows, dc * P:(dc + 1) * P],
                                identb[0:rows, 0:rows])
            nc.vector.tensor_copy(imgsT[dc][:, c0:c0 + rows], pst2[:, 0:rows])

    pg2_pool.release()
    pmsg_pool.release()
    ptr2_pool.release()
    ptrv_pool.release()
    gpool.release()
    v_pool.release()

    # ----- map MLP -----
    pmm2 = tc.alloc_tile_pool(name="pmm2", bufs=6, space="PSUM")
    hm_pool = tc.alloc_tile_pool(name="hm", bufs=1, side="left")
    hm = mlp("hm", T["mw1"], KC, btiles["mb1"], AF.Relu, imgsT, hm_pool, MC, pmm2)
    imgsT_pool.release()
    emb_pool = tc.alloc_tile_pool(name="emb", bufs=1, side="right")
    embT = mlp("embT", T["mw2"], KC, btiles["mb2"], AF.Identity, hm, emb_pool,
               ECH, pmm2, out_dtype=F32)
    hm_pool.release()
    pmm2.release()

    # ----- final l2norm in row-major space, write row-major output -----
    pet_pool = tc.alloc_tile_pool(name="pet", bufs=2, space="PSUM")
    fin_pool = tc.alloc_tile_pool(name="fin", bufs=2, side="left")
    for g, (i0, ng) in enumerate(GROUPS):
        rows = ng * K
        c0 = i0 * K
        embR = fin_pool.tile([108, E], F32, name=f"embR{g}", tag="embR")
        for ec in range(ECH):
            pet = pet_pool.tile([108, P], F32, name=f"pet{g}_{ec}", tag="pet")
            nc.tensor.transpose(pet[0:rows, :], embT[ec][:, c0:c0 + rows],
                                identf[:])
            nc.scalar.copy(embR[0:rows, ec * P:(ec + 1) * P], pet[0:rows, :])
        sqf = fin_pool.tile([108, 512], F32, name=f"sqf{g}", tag="sqf")
        nsqf = fin_pool.tile([108, 8], F32, name=f"nsqf{g}", tag="nsqf")
        for q in range(2):
            sl = slice(q * 512, (q + 1) * 512)
            nc.vector.tensor_mul(sqf[0:rows, :], embR[0:rows, sl], embR[0:rows, sl])
            nc.vector.tensor_reduce(nsqf[0:rows, q:q + 1], sqf[0:rows, :],
                                    axis=mybir.AxisListType.X, op=OP.add)
        nc.vector.tensor_add(nsqf[0:rows, 2:3], nsqf[0:rows, 0:1],
                             nsqf[0:rows, 1:2])
        nc.scalar.sqrt(nsqf[0:rows, 3:4], nsqf[0:rows, 2:3])
        nc.vector.tensor_scalar_add(nsqf[0:rows, 4:5], nsqf[0:rows, 3:4], 1e-8)
        nc.vector.reciprocal(nsqf[0:rows, 5:6], nsqf[0:rows, 4:5])
        embO = fin_pool.tile([108, E], F32, name=f"embO{g}", tag="embO")
        nc.vector.tensor_scalar_mul(embO[0:rows, :], embR[0:rows, :],
                                    nsqf[0:rows, 5:6])
        nc.sync.dma_start(T["outp"][c0:c0 + rows, :], embO[0:rows, :])
    pet_pool.release()
    fin_pool.release()
    emb_pool.release()
    misc.release()
    wpool.release()
    const.release()


def build_program(loop=1):
    nc = bacc.Bacc("TRN2", target_bir_lowering=False, debug=False,
                   num_devices=NCORES)
    T = _declare(nc)
    with tile.TileContext(nc) as tc:
        for _ in range(loop):
            _emit(nc, tc, T)
    nc.compile()
    return nc


# ------------------------------------------------------------- host glue ---

def _packw_aligned(w, nkc, dtype=NPBF16, scale=1.0):
    """(Kdim, M) fp32 -> (M/128, 128, nkc, 128)."""
    m = w.shape[1]
    mc = m // 128
    wp = np.asarray(w, np.float32) * scale
    if dtype is NPFP8:
        wp = np.clip(wp, -240.0, 240.0)
    return np.ascontiguousarray(
        wp.reshape(nkc, 128, mc, 128).transpose(2, 1, 0, 3)
    ).astype(dtype)


def _packw_x(w, dtype=NPBF16, scale=1.0):
    """(2053, M) fp32 -> 18-chunk layout: img dims 0..2047, bbox dims in
    chunk 16 rows 0..3, area dim in chunk 17 row 0."""
    m = w.shape[1]
    wp = np.zeros((KC1 * 128, m), np.float32)
    wp[:2048] = w[:2048]
    wp[2048:2052] = w[2048:2052]     # chunk 16, rows 0..3
    wp[17 * 128] = w[2052]           # chunk 17, row 0
    return _packw_aligned(wp, KC1, dtype=dtype, scale=scale)


def _bias(b):
    return np.ascontiguousarray(np.asarray(b, np.float32).reshape(-1, 128).T)


def prepare_inputs(inputs):
    images = np.asarray(inputs["images"], np.float32)
    bboxes = np.asarray(inputs["bboxes"], np.float32)
    img_range = np.asarray(inputs["img_range"], np.float32)

    sel = np.array([[-1.0, 0.0], [0.0, -1.0], [1.0, 0.0], [0.0, 1.0]], np.float32)
    Sb = np.zeros((128, 108), np.float32)
    for j in range(108):
        Sb[j % K, j] = 1.0
    bmask = np.zeros((108, 108), np.float32)
    for blk in range(3):
        bmask[blk * K:(blk + 1) * K, blk * K:(blk + 1) * K] = 1.0
    L36 = np.vstack([np.triu(np.ones((K, K), np.float32)),
                     np.zeros((128 - K, K), np.float32)])

    shared = {
        "gw1": _packw_x(np.asarray(inputs["gate_w1"], np.float32),
                        dtype=NPFP8, scale=WSCALE),
        "nw1": _packw_x(np.asarray(inputs["node_w1"], np.float32),
                        dtype=NPFP8, scale=WSCALE),
        "nw2": _packw_aligned(np.asarray(inputs["node_w2"], np.float32), KC,
                              dtype=NPFP8, scale=WSCALE),
        "mw1": _packw_aligned(np.asarray(inputs["map_w1"], np.float32), KC),
        "mw2": _packw_aligned(np.asarray(inputs["map_w2"], np.float32), KC),
        "gw2": np.ascontiguousarray(
            WSCALE * np.asarray(inputs["gate_w2"], np.float32).reshape(MC, 128).T
        ).astype(NPFP8),
        "gb1": _bias(inputs["gate_b1"]),
        "nb1": _bias(inputs["node_b1"]),
        "nb2": _bias(inputs["node_b2"]),
        "mb1": _bias(inputs["map_b1"]),
        "mb2": _bias(inputs["map_b2"]),
        "gb2": np.asarray(inputs["gate_b2"], np.float32).reshape(1, 1),
        "L36": L36.astype(NPBF16),
        "sel": sel.astype(NPBF16),
        "Sb": Sb.astype(NPBF16),
        "bmask": bmask.astype(NPBF16),
    }

    in_maps = []
    for c in range(NCORES):
        sl = slice(c * BSH, (c + 1) * BSH)
        imf = images[sl].reshape(R, D)
        rt = np.zeros((128, R), np.float32)
        rt[:K] = img_range[sl].transpose(2, 0, 1).reshape(K, R)
        m = dict(shared)
        m["imT"] = np.ascontiguousarray(
            imf.T.reshape(KC, 128, R).transpose(1, 0, 2)).astype(NPFP8)
        m["im"] = imf.astype(NPBF16)
        m["bbT"] = np.ascontiguousarray(bboxes[sl].reshape(R, 4).T).astype(NPBF16)
        m["rT"] = rt.astype(NPBF16)
        in_maps.append(m)
    return in_maps


def run(inputs, trace=False):
    nc = build_program()
    in_maps = prepare_inputs(inputs)
    res = run_bass_kernel_spmd(nc, in_maps, list(range(NCORES)), trace=trace)
    out = np.empty((B, K, E), np.float32)
    for c in range(NCORES):
        out[c * BSH:(c + 1) * BSH] = res.results[c]["outp"].reshape(BSH, K, E)
    return out, res


def kernel(**inputs):
    out, _ = run(inputs, trace=False)
    return out



# revision 7
# speedup vs baseline: 8.7949x; 8.7949x over previous
"""Trainium2 Bass kernel for nn_EncoderImage (gnn_message_passing).

Strategy: pure data-parallel over batch (32 images/core x 8 cores).
All MLP math runs feature-major ([feature_partitions, row_free]) so weight
tiles DMA directly as matmul lhsT and biases are per-partition ACT operands.

The topk/gather message passing is rewritten exactly (img_range values are
only 0/1, and jax.lax.top_k tie-breaks by lowest index):
    out[b] = (W0[b] + diag(relu(5 - c[b]))) @ (m[b] * v[b])
where W0[b][k,j] = r[k,j] * (cumsum_j r[k,:] <= 5) and c[b][k] = sum_j r[k,j].
The cumsum is a matmul against a constant triangular matrix; per 3-image
group the 36x36 W' blocks are assembled block-diagonally by a selector
matmul + block mask, so the whole group is one K=108 stationary operand.

Matmuls are bf16 (inputs rounded; fp32 PSUM accumulation); norms/biases fp32.

Hardware constraints honored: engine APs start at partition 0/32/64/96;
at most one PSUM input per DVE op; no tensor_tensor_reduce.
"""

import numpy as np
import ml_dtypes

import concourse.bacc as bacc
import concourse.bass as bass
import concourse.tile as tile
import concourse.mybir as mybir
from concourse.bass_utils import run_bass_kernel_spmd
from concourse.masks import make_identity

F32 = mybir.dt.float32
BF16 = mybir.dt.bfloat16
FP8 = mybir.dt.float8e4
AF = mybir.ActivationFunctionType
OP = mybir.AluOpType
NPBF16 = ml_dtypes.bfloat16
NPFP8 = ml_dtypes.float8_e4m3
WSCALE = 16.0

B, K, D, E = 256, 36, 2048, 1024
NCORES = 8
BSH = B // NCORES           # 32 images per core
R = BSH * K                 # 1152 rows per core
NT, NW = 3, 384             # row windows per psum accumulation group
KC1 = 18                    # k-chunks for x (2048 img + 4 bbox + 1 area)
KC = 16                     # k-chunks for D
MC = 16                     # m-chunks for D outputs
ECH = 8                     # m-chunks for E outputs
GROUPS = [(i, 3) for i in range(0, 30, 3)] + [(30, 2)]   # (start_img, n_imgs)


# ---------------------------------------------------------------- program ---

def _declare(nc):
    t = {}
    def inp(name, shape, dt):
        t[name] = nc.dram_tensor(name, list(shape), dt, kind="ExternalInput").ap()
    inp("imT", (128, KC, R), FP8)
    inp("im", (R, D), BF16)
    inp("bbT", (4, R), BF16)
    inp("rT", (128, R), BF16)
    inp("L36", (128, K), BF16)
    inp("sel", (4, 2), BF16)
    inp("Sb", (128, 108), BF16)
    inp("bmask", (108, 108), BF16)
    inp("gw1", (MC, 128, KC1, 128), FP8)
    inp("nw1", (MC, 128, KC1, 128), FP8)
    inp("nw2", (MC, 128, KC, 128), FP8)
    inp("mw1", (MC, 128, KC, 128), BF16)
    inp("mw2", (ECH, 128, KC, 128), BF16)
    inp("gw2", (128, MC), FP8)
    inp("gb1", (128, MC), F32)
    inp("nb1", (128, MC), F32)
    inp("nb2", (128, MC), F32)
    inp("mb1", (128, MC), F32)
    inp("mb2", (128, ECH), F32)
    inp("gb2", (1, 1), F32)
    t["outp"] = nc.dram_tensor("outp", [R, E], F32, kind="ExternalOutput").ap()
    return t


def _emit(nc, tc, T):
    P = 128

    # ----- whole-kernel pools (left side) -----
    const = tc.alloc_tile_pool(name="const", bufs=1, side="left")
    wpool = tc.alloc_tile_pool(name="wts", bufs=2, side="left")
    misc = tc.alloc_tile_pool(name="misc", bufs=1, side="left")

    identb = const.tile([P, P], BF16, name="identb", tag="identb")
    make_identity(nc, identb)
    identf = const.tile([P, P], F32, name="identf", tag="identf")
    make_identity(nc, identf)
    ones_b = const.tile([P, 1], BF16, name="ones_b", tag="ones_b")
    nc.vector.memset(ones_b[:], 1.0)
    five_t = const.tile([1, 1], F32, name="five_t", tag="five_t")
    nc.vector.memset(five_t[:], 5.0)
    L36t = const.tile([P, K], BF16, name="L36t", tag="L36t")
    nc.sync.dma_start(L36t[:], T["L36"][:])
    selt = const.tile([4, 2], BF16, name="selt", tag="selt")
    nc.sync.dma_start(selt[:], T["sel"][:])
    Sbt = const.tile([P, 108], BF16, name="Sbt", tag="Sbt")
    nc.sync.dma_start(Sbt[:], T["Sb"][:])
    bmaskt = const.tile([108, 108], BF16, name="bmaskt", tag="bmaskt")
    nc.sync.dma_start(bmaskt[:], T["bmask"][:])
    rTt = const.tile([P, R], BF16, name="rTt", tag="rTt")
    nc.sync.dma_start(rTt[:], T["rT"][:])
    gw2t = const.tile([P, MC], FP8, name="gw2t", tag="gw2t")
    nc.sync.dma_start(gw2t[:], T["gw2"][:])
    btiles = {}
    for bn, cols in (("gb1", MC), ("nb1", MC), ("nb2", MC), ("mb1", MC), ("mb2", ECH)):
        bt = const.tile([P, cols], F32, name=bn + "t", tag=bn + "t")
        nc.sync.dma_start(bt[:], T[bn][:])
        btiles[bn] = bt
    gb2t = const.tile([1, 1], F32, name="gb2t", tag="gb2t")
    nc.sync.dma_start(gb2t[:], T["gb2"][:])
    bbT = misc.tile([4, R], BF16, name="bbT", tag="bbT")
    nc.sync.dma_start(bbT[:], T["bbT"][:])

    # ----- xT build: one [128, 18, R] fp8 tile -----
    pd_pool = tc.alloc_tile_pool(name="pd", bufs=2, space="PSUM")
    xpool = tc.alloc_tile_pool(name="xT", bufs=1, side="left")
    xTall = xpool.tile([P, KC1, R], FP8, name="xTall", tag="xTall")
    nc.sync.dma_start(xTall[:, 0:KC, :], T["imT"][:])
    nc.vector.memset(xTall[:, KC:KC1, :], 0.0)
    nc.vector.tensor_scalar_mul(xTall[0:4, KC, :], bbT[0:4, :], 0.1)
    d1s = misc.tile([1, R], F32, name="d1s", tag="d1s")
    for n in range(NT):
        sl = slice(n * NW, (n + 1) * NW)
        pd1 = pd_pool.tile([1, NW], F32, name=f"pd1_{n}", tag="pd1")
        nc.tensor.matmul(pd1[:], selt[:, 0:1], bbT[:, sl], start=True, stop=True)
        pd2 = pd_pool.tile([1, NW], F32, name=f"pd2_{n}", tag="pd2")
        nc.tensor.matmul(pd2[:], selt[:, 1:2], bbT[:, sl], start=True, stop=True)
        nc.scalar.copy(d1s[0:1, sl], pd1[:])
        # area*0.1 = (d2 * 0.1) * d1
        nc.vector.scalar_tensor_tensor(xTall[0:1, KC1 - 1, sl], pd2[:], 0.1,
                                       d1s[0:1, sl], OP.mult, OP.mult)
    pd_pool.release()

    # ----- MLP layer helpers (feature-major) -----
    DR = mybir.MatmulPerfMode.DoubleRow

    def mlp(lname, wap, nkc, bias_tile, func, rhs, out_pool, mcs, pspool,
            out_dtype=BF16):
        """bf16 layer: rhs is a list of [128, R] tiles."""
        outs = []
        for mc in range(mcs):
            wt = wpool.tile([P, nkc, P], BF16, name=f"w_{lname}_{mc}", tag="wstream",
                            padded_shape=[P, KC1, P])
            nc.sync.dma_start(wt[:], wap[mc])
            ot = out_pool.tile([P, R], out_dtype, name=f"{lname}_{mc}",
                               tag=f"{lname}_{mc}")
            for n in range(NT):
                ps = pspool.tile([P, NW], F32, name=f"ps_{lname}_{mc}_{n}", tag="pmm")
                for kc in range(nkc):
                    nc.tensor.matmul(ps[:], wt[:, kc, :],
                                     rhs[kc][:, n * NW:(n + 1) * NW],
                                     start=(kc == 0), stop=(kc == nkc - 1))
                nc.scalar.activation(ot[:, n * NW:(n + 1) * NW], ps[:], func,
                                     bias=bias_tile[:, mc:mc + 1])
            outs.append(ot)
        return outs

    def mlp8(lname, wap, nkc, bias_tile, func, rhs_all, mcs, pspool,
             out_all=None, out_pool=None, out_dtype=BF16):
        """fp8 DoubleRow layer: rhs_all is one [128, nkc, R] fp8 tile;
        weights are pre-scaled by WSCALE, descaled in the ACT eviction.
        Writes into big tile out_all [128, mcs, R] if given, else returns
        a list of [128, R] tiles from out_pool."""
        outs = []
        for mc in range(mcs):
            wt = wpool.tile([P, nkc, P], FP8, name=f"w_{lname}_{mc}", tag="wstream8",
                            padded_shape=[P, KC1, P])
            nc.sync.dma_start(wt[:], wap[mc])
            if out_all is None:
                ot = out_pool.tile([P, R], out_dtype, name=f"{lname}_{mc}",
                                   tag=f"{lname}_{mc}")
                outs.append(ot)
            for n in range(NT):
                nsl = slice(n * NW, (n + 1) * NW)
                ps = pspool.tile([P, NW], F32, name=f"ps_{lname}_{mc}_{n}", tag="pmm")
                for kc in range(0, nkc, 2):
                    nc.tensor.matmul(ps[:], wt[:, kc:kc + 2, :],
                                     rhs_all[:, kc:kc + 2, nsl],
                                     start=(kc == 0), stop=(kc == nkc - 2),
                                     perf_mode=DR)
                dst = out_all[:, mc, nsl] if out_all is not None else ot[:, nsl]
                nc.scalar.activation(dst, ps[:], func, scale=1.0 / WSCALE,
                                     bias=bias_tile[:, mc:mc + 1])
        return outs

    # ----- gate MLP -----
    pmm = tc.alloc_tile_pool(name="pmm", bufs=6, space="PSUM")
    psg_pool = tc.alloc_tile_pool(name="psg", bufs=2, space="PSUM")
    hg_pool = tc.alloc_tile_pool(name="hg", bufs=1, side="left")
    hgall = hg_pool.tile([P, MC, R], FP8, name="hgall", tag="hgall")
    mlp8("hg", T["gw1"], KC1, btiles["gb1"], AF.Relu, xTall, MC, pmm,
         out_all=hgall)

    m_row = misc.tile([32, R], F32, name="m_row", tag="m_row")
    nc.vector.memset(m_row[:], 0.0)
    for n in range(NT):
        psg = psg_pool.tile([1, NW], F32, name=f"psg{n}", tag="psg")
        for kc in range(MC):
            nc.tensor.matmul(psg[:], gw2t[:, kc:kc + 1],
                             hgall[:, kc, n * NW:(n + 1) * NW],
                             start=(kc == 0), stop=(kc == MC - 1))
        nc.scalar.activation(m_row[0:1, n * NW:(n + 1) * NW], psg[:], AF.Sigmoid,
                             scale=1.0 / WSCALE, bias=gb2t[0:1, 0:1])
    psg_pool.release()
    hg_pool.release()

    # ----- W0 build (cumsum mask) + w = relu(5 - c) -----
    psc_pool = tc.alloc_tile_pool(name="psc", bufs=1, space="PSUM")
    w0a = misc.tile([P, R], BF16, name="w0a", tag="w0a")
    nc.vector.memset(w0a[:], 0.0)
    w_row = misc.tile([32, R], F32, name="w_row", tag="w_row")
    nc.vector.memset(w_row[:], 0.0)
    for n in range(NT):
        sl = slice(n * NW, (n + 1) * NW)
        psc = psc_pool.tile([K, NW], F32, name=f"psc{n}", tag="psc")
        nc.tensor.matmul(psc[:], L36t[:], rTt[:, sl], start=True, stop=True)
        nc.vector.scalar_tensor_tensor(w0a[0:K, sl], psc[:], 5.0, rTt[0:K, sl],
                                       OP.is_le, OP.mult)
        pcc = psc_pool.tile([1, NW], F32, name=f"pcc{n}", tag="pcc")
        nc.tensor.matmul(pcc[:], ones_b[:], rTt[:, sl], start=True, stop=True)
        nc.scalar.activation(w_row[0:1, sl], pcc[:], AF.Relu,
                             bias=five_t[0:1, 0:1], scale=-1.0)
    psc_pool.release()

    # ----- node MLP -----
    hn_pool = tc.alloc_tile_pool(name="hn", bufs=1, side="right")
    hnall = hn_pool.tile([P, MC, R], FP8, name="hnall", tag="hnall")
    mlp8("hn", T["nw1"], KC1, btiles["nb1"], AF.Relu, xTall, MC, pmm,
         out_all=hnall)
    xpool.release()
    v_pool = tc.alloc_tile_pool(name="v", bufs=1, side="left")
    v = mlp8("v", T["nw2"], KC, btiles["nb2"], AF.Identity, hnall, MC, pmm,
             out_pool=v_pool)
    hn_pool.release()
    pmm.release()

    # ----- message passing, per image group -----
    imgsT_pool = tc.alloc_tile_pool(name="imgsT", bufs=1, side="right")
    gpool = tc.alloc_tile_pool(name="gp", bufs=2, side="right")
    ptrv_pool = tc.alloc_tile_pool(name="ptrv", bufs=3, space="PSUM")
    ptr2_pool = tc.alloc_tile_pool(name="ptr2", bufs=2, space="PSUM")
    pmsg_pool = tc.alloc_tile_pool(name="pmsg", bufs=2, space="PSUM")
    pg2_pool = tc.alloc_tile_pool(name="pg2", bufs=1, space="PSUM")
    imgsTall = imgsT_pool.tile([P, MC, R], FP8, name="imgsTall", tag="imgsTall")

    for g, (i0, ng) in enumerate(GROUPS):
        rows = ng * K
        c0 = i0 * K

        # per-row scalars: mT (sigmoid gate) and wT (self-fallback count)
        pmw1 = ptrv_pool.tile([108, 32], F32, name=f"pmw1_{g}", tag="ptrv")
        nc.tensor.transpose(pmw1[0:rows, :], m_row[:, c0:c0 + rows],
                            identf[0:32, 0:32])
        pmw2 = ptrv_pool.tile([108, 32], F32, name=f"pmw2_{g}", tag="ptrv")
        nc.tensor.transpose(pmw2[0:rows, :], w_row[:, c0:c0 + rows],
                            identf[0:32, 0:32])
        mwT = gpool.tile([108, 2], F32, name=f"mwT{g}", tag="mwT")
        nc.vector.tensor_copy(mwT[0:rows, 0:1], pmw1[0:rows, 0:1])
        nc.vector.tensor_copy(mwT[0:rows, 1:2], pmw2[0:rows, 0:1])

        # block-diagonal stationary operand gW = blockdiag(W0^T) + diag(wT)
        pg2 = pg2_pool.tile([108, 108], F32, name=f"pg2_{g}", tag="pg2")
        nc.tensor.matmul(pg2[0:rows, 0:rows], Sbt[:, 0:rows],
                         w0a[:, c0:c0 + rows], start=True, stop=True)
        gW = gpool.tile([108, 108], BF16, name=f"gW{g}", tag="gW")
        nc.vector.tensor_mul(gW[0:rows, 0:rows], pg2[0:rows, 0:rows],
                             bmaskt[0:rows, 0:rows])
        tmpd = gpool.tile([108, 108], BF16, name=f"tmpd{g}", tag="tmpd")
        nc.vector.tensor_scalar_mul(tmpd[0:rows, 0:rows], identb[0:rows, 0:rows],
                                    mwT[0:rows, 1:2])
        nc.vector.tensor_add(gW[0:rows, 0:rows], gW[0:rows, 0:rows],
                             tmpd[0:rows, 0:rows])

        # u = m * v, transposed to row-major [rows, D]
        u = gpool.tile([108, D], BF16, name=f"u{g}", tag="u")
        for dc in range(MC):
            psv = ptrv_pool.tile([108, P], BF16, name=f"psv{g}_{dc}", tag="ptrv")
            nc.tensor.transpose(psv[0:rows, :], v[dc][:, c0:c0 + rows], identb[:])
            nc.vector.tensor_scalar_mul(u[0:rows, dc * P:(dc + 1) * P],
                                        psv[0:rows, :], mwT[0:rows, 0:1])

        # out = gW^T @ u  [rows, D], evicted to SBUF via ACT
        out_sb = gpool.tile([108, D], F32, name=f"outsb{g}", tag="outsb")
        for q in range(4):
            sl = slice(q * 512, (q + 1) * 512)
            psm = pmsg_pool.tile([108, 512], F32, name=f"psm{g}_{q}", tag="pmsg")
            nc.tensor.matmul(psm[0:rows, :], gW[0:rows, 0:rows], u[0:rows, sl],
                             start=True, stop=True)
            nc.scalar.copy(out_sb[0:rows, sl], psm[0:rows, :])

        # l2norm + residual add with original images
        sq = gpool.tile([108, 512], F32, name=f"sq{g}", tag="sq")
        nsq = gpool.tile([108, 8], F32, name=f"nsq{g}", tag="nsq")
        for q in range(4):
            sl = slice(q * 512, (q + 1) * 512)
            nc.vector.tensor_mul(sq[0:rows, :], out_sb[0:rows, sl],
                                 out_sb[0:rows, sl])
            nc.vector.tensor_reduce(nsq[0:rows, q:q + 1], sq[0:rows, :],
                                    axis=mybir.AxisListType.X, op=OP.add)
        nc.vector.tensor_add(nsq[0:rows, 4:5], nsq[0:rows, 0:1], nsq[0:rows, 1:2])
        nc.vector.tensor_add(nsq[0:rows, 5:6], nsq[0:rows, 2:3], nsq[0:rows, 3:4])
        nc.vector.tensor_add(nsq[0:rows, 6:7], nsq[0:rows, 4:5], nsq[0:rows, 5:6])
        nc.scalar.sqrt(nsq[0:rows, 7:8], nsq[0:rows, 6:7])
        inv = gpool.tile([108, 2], F32, name=f"inv{g}", tag="inv")
        nc.vector.tensor_scalar_add(inv[0:rows, 0:1], nsq[0:rows, 7:8], 1e-8)
        nc.vector.reciprocal(inv[0:rows, 1:2], inv[0:rows, 0:1])

        img_ld = gpool.tile([108, D], BF16, name=f"img{g}", tag="img")
        nc.sync.dma_start(img_ld[0:rows, :], T["im"][c0:c0 + rows, :])
        imgsw = gpool.tile([108, D], BF16, name=f"imgsw{g}", tag="imgsw")
        for q in range(4):
            sl = slice(q * 512, (q + 1) * 512)
            nc.vector.scalar_tensor_tensor(imgsw[0:rows, sl], out_sb[0:rows, sl],
                                           inv[0:rows, 1:2], img_ld[0:rows, sl],
                                           OP.mult, OP.add)

        # transpose back to feature-major imgsT
        for dc in range(MC):
            pst2 = ptr2_pool.tile([P, 108], BF16, name=f"pst2_{g}_{dc}", tag="ptr2")
            nc.tensor.transpose(pst2[:, 0:rows], imgsw[0:rows, dc * P:(dc + 1) * P],
                                identb[0:rows, 0:rows])
            nc.vector.tensor_copy(imgsTall[:, dc, c0:c0 + rows], pst2[:, 0:rows])

    pg2_pool.release()
    pmsg_pool.release()
    ptr2_pool.release()
    ptrv_pool.release()
    gpool.release()
    v_pool.release()

    # ----- map MLP (fp8 DoubleRow) -----
    pmm2 = tc.alloc_tile_pool(name="pmm2", bufs=6, space="PSUM")
    hm_pool = tc.alloc_tile_pool(name="hm", bufs=1, side="left")
    hmall = hm_pool.tile([P, MC, R], FP8, name="hmall", tag="hmall")
    mlp8("hm", T["mw1"], KC, btiles["mb1"], AF.Relu, imgsTall, MC, pmm2,
         out_all=hmall)
    imgsT_pool.release()
    emb_pool = tc.alloc_tile_pool(name="emb", bufs=1, side="right")
    embT = mlp8("embT", T["mw2"], KC, btiles["mb2"], AF.Identity, hmall, ECH,
                pmm2, out_pool=emb_pool, out_dtype=F32)
    hm_pool.release()
    pmm2.release()

    # ----- final l2norm in row-major space, write row-major output -----
    pet_pool = tc.alloc_tile_pool(name="pet", bufs=2, space="PSUM")
    fin_pool = tc.alloc_tile_pool(name="fin", bufs=2, side="left")
    for g, (i0, ng) in enumerate(GROUPS):
        rows = ng * K
        c0 = i0 * K
        embR = fin_pool.tile([108, E], F32, name=f"embR{g}", tag="embR")
        for ec in range(ECH):
            pet = pet_pool.tile([108, P], F32, name=f"pet{g}_{ec}", tag="pet")
            nc.tensor.transpose(pet[0:rows, :], embT[ec][:, c0:c0 + rows],
                                identf[:])
            nc.scalar.copy(embR[0:rows, ec * P:(ec + 1) * P], pet[0:rows, :])
        sqf = fin_pool.tile([108, 512], F32, name=f"sqf{g}", tag="sqf")
        nsqf = fin_pool.tile([108, 8], F32, name=f"nsqf{g}", tag="nsqf")
        for q in range(2):
            sl = slice(q * 512, (q + 1) * 512)
            nc.vector.tensor_mul(sqf[0:rows, :], embR[0:rows, sl], embR[0:rows, sl])
            nc.vector.tensor_reduce(nsqf[0:rows, q:q + 1], sqf[0:rows, :],
                                    axis=mybir.AxisListType.X, op=OP.add)
        nc.vector.tensor_add(nsqf[0:rows, 2:3], nsqf[0:rows, 0:1],
                             nsqf[0:rows, 1:2])
        nc.scalar.sqrt(nsqf[0:rows, 3:4], nsqf[0:rows, 2:3])
        nc.vector.tensor_scalar_add(nsqf[0:rows, 4:5], nsqf[0:rows, 3:4], 1e-8)
        nc.vector.reciprocal(nsqf[0:rows, 5:6], nsqf[0:rows, 4:5])
        embO = fin_pool.tile([108, E], F32, name=f"embO{g}", tag="embO")
        nc.vector.tensor_scalar_mul(embO[0:rows, :], embR[0:rows, :],
                                    nsqf[0:rows, 5:6])
        nc.sync.dma_start(T["outp"][c0:c0 + rows, :], embO[0:rows, :])
    pet_pool.release()
    fin_pool.release()
    emb_pool.release()
    misc.release()
    wpool.release()
    const.release()


def build_program(loop=1):
    nc = bacc.Bacc("TRN2", target_bir_lowering=False, debug=False,
                   num_devices=NCORES)
    T = _declare(nc)
    with tile.TileContext(nc) as tc:
        for _ in range(loop):
            _emit(nc, tc, T)
    nc.compile()
    return nc


# ------------------------------------------------------------- host glue ---

def _packw_aligned(w, nkc, dtype=NPBF16, scale=1.0):
    """(Kdim, M) fp32 -> (M/128, 128, nkc, 128)."""
    m = w.shape[1]
    mc = m // 128
    wp = np.asarray(w, np.float32) * scale
    if dtype is NPFP8:
        wp = np.clip(wp, -240.0, 240.0)
    return np.ascontiguousarray(
        wp.reshape(nkc, 128, mc, 128).transpose(2, 1, 0, 3)
    ).astype(dtype)


def _packw_x(w, dtype=NPBF16, scale=1.0):
    """(2053, M) fp32 -> 18-chunk layout: img dims 0..2047, bbox dims in
    chunk 16 rows 0..3, area dim in chunk 17 row 0."""
    m = w.shape[1]
    wp = np.zeros((KC1 * 128, m), np.float32)
    wp[:2048] = w[:2048]
    wp[2048:2052] = w[2048:2052]     # chunk 16, rows 0..3
    wp[17 * 128] = w[2052]           # chunk 17, row 0
    return _packw_aligned(wp, KC1, dtype=dtype, scale=scale)


def _bias(b):
    return np.ascontiguousarray(np.asarray(b, np.float32).reshape(-1, 128).T)


def prepare_inputs(inputs):
    images = np.asarray(inputs["images"], np.float32)
    bboxes = np.asarray(inputs["bboxes"], np.float32)
    img_range = np.asarray(inputs["img_range"], np.float32)

    sel = np.array([[-1.0, 0.0], [0.0, -1.0], [1.0, 0.0], [0.0, 1.0]], np.float32)
    Sb = np.zeros((128, 108), np.float32)
    for j in range(108):
        Sb[j % K, j] = 1.0
    bmask = np.zeros((108, 108), np.float32)
    for blk in range(3):
        bmask[blk * K:(blk + 1) * K, blk * K:(blk + 1) * K] = 1.0
    L36 = np.vstack([np.triu(np.ones((K, K), np.float32)),
                     np.zeros((128 - K, K), np.float32)])

    shared = {
        "gw1": _packw_x(np.asarray(inputs["gate_w1"], np.float32),
                        dtype=NPFP8, scale=WSCALE),
        "nw1": _packw_x(np.asarray(inputs["node_w1"], np.float32),
                        dtype=NPFP8, scale=WSCALE),
        "nw2": _packw_aligned(np.asarray(inputs["node_w2"], np.float32), KC,
                              dtype=NPFP8, scale=WSCALE),
        "mw1": _packw_aligned(np.asarray(inputs["map_w1"], np.float32), KC),
        "mw2": _packw_aligned(np.asarray(inputs["map_w2"], np.float32), KC),
        "gw2": np.ascontiguousarray(
            WSCALE * np.asarray(inputs["gate_w2"], np.float32).reshape(MC, 128).T
        ).astype(NPFP8),
        "gb1": _bias(inputs["gate_b1"]),
        "nb1": _bias(inputs["node_b1"]),
        "nb2": _bias(inputs["node_b2"]),
        "mb1": _bias(inputs["map_b1"]),
        "mb2": _bias(inputs["map_b2"]),
        "gb2": np.asarray(inputs["gate_b2"], np.float32).reshape(1, 1),
        "L36": L36.astype(NPBF16),
        "sel": sel.astype(NPBF16),
        "Sb": Sb.astype(NPBF16),
        "bmask": bmask.astype(NPBF16),
    }

    in_maps = []
    for c in range(NCORES):
        sl = slice(c * BSH, (c + 1) * BSH)
        imf = images[sl].reshape(R, D)
        rt = np.zeros((128, R), np.float32)
        rt[:K] = img_range[sl].transpose(2, 0, 1).reshape(K, R)
        m = dict(shared)
        m["imT"] = np.ascontiguousarray(
            imf.T.reshape(KC, 128, R).transpose(1, 0, 2)).astype(NPFP8)
        m["im"] = imf.astype(NPBF16)
        m["bbT"] = np.ascontiguousarray(bboxes[sl].reshape(R, 4).T).astype(NPBF16)
        m["rT"] = rt.astype(NPBF16)
        in_maps.append(m)
    return in_maps


def run(inputs, trace=False):
    nc = build_program()
    in_maps = prepare_inputs(inputs)
    res = run_bass_kernel_spmd(nc, in_maps, list(range(NCORES)), trace=trace)
    out = np.empty((B, K, E), np.float32)
    for c in range(NCORES):
        out[c * BSH:(c + 1) * BSH] = res.results[c]["outp"].reshape(BSH, K, E)
    return out, res


def kernel(**inputs):
    out, _ = run(inputs, trace=False)
    return out



# revision 13
# speedup vs baseline: 9.6664x; 1.0991x over previous
"""Trainium2 Bass kernel for nn_EncoderImage (gnn_message_passing).

Strategy: pure data-parallel over batch (32 images/core x 8 cores).
All MLP math runs feature-major ([feature_partitions, row_free]) so weight
tiles DMA directly as matmul lhsT and biases are per-partition ACT operands.

The topk/gather message passing is rewritten exactly (img_range values are
only 0/1, and jax.lax.top_k tie-breaks by lowest index):
    out[b] = (W0[b] + diag(relu(5 - c[b]))) @ (m[b] * v[b])
where W0[b][k,j] = r[k,j] * (cumsum_j r[k,:] <= 5) and c[b][k] = sum_j r[k,j].
The cumsum is a matmul against a constant triangular matrix; per 3-image
group the 36x36 W' blocks are assembled block-diagonally by a selector
matmul + block mask, so the whole group is one K=108 stationary operand.

Matmuls are bf16 (inputs rounded; fp32 PSUM accumulation); norms/biases fp32.

Hardware constraints honored: engine APs start at partition 0/32/64/96;
at most one PSUM input per DVE op; no tensor_tensor_reduce.
"""

import numpy as np
import ml_dtypes

import concourse.bacc as bacc
import concourse.bass as bass
import concourse.tile as tile
import concourse.mybir as mybir
from concourse.bass_utils import run_bass_kernel_spmd
from concourse.masks import make_identity

F32 = mybir.dt.float32
BF16 = mybir.dt.bfloat16
FP8 = mybir.dt.float8e4
AF = mybir.ActivationFunctionType
OP = mybir.AluOpType
NPBF16 = ml_dtypes.bfloat16
NPFP8 = ml_dtypes.float8_e4m3
WSCALE = 16.0

B, K, D, E = 256, 36, 2048, 1024
NCORES = 8
BSH = B // NCORES           # 32 images per core
R = BSH * K                 # 1152 rows per core
NT, NW = 3, 384             # row windows per psum accumulation group
KC1 = 18                    # k-chunks for x (2048 img + 4 bbox + 1 area)
KC = 16                     # k-chunks for D
MC = 16                     # m-chunks for D outputs
ECH = 8                     # m-chunks for E outputs
GROUPS = [(i, 3) for i in range(0, 30, 3)] + [(30, 2)]   # (start_img, n_imgs)


# ---------------------------------------------------------------- program ---

def _declare(nc):
    t = {}
    def inp(name, shape, dt):
        t[name] = nc.dram_tensor(name, list(shape), dt, kind="ExternalInput").ap()
    inp("imT", (128, KC, R), FP8)
    inp("im", (R, D), BF16)
    inp("bbT", (4, R), BF16)
    inp("rT", (128, R), BF16)
    inp("L36", (128, K), BF16)
    inp("sel", (4, 2), BF16)
    inp("Sb", (128, 108), BF16)
    inp("bmask", (108, 108), BF16)
    inp("gw1", (MC, 128, KC1, 128), FP8)
    inp("nw1", (MC, 128, KC1, 128), FP8)
    inp("nw2m", (128, KC, 2048), FP8)
    inp("nb2r", (1, 2048), BF16)
    inp("mw1", (MC, 128, KC, 128), BF16)
    inp("mw2", (ECH, 128, KC, 128), BF16)
    inp("gw2", (128, MC), FP8)
    inp("gb1", (128, MC), F32)
    inp("nb1", (128, MC), F32)
    inp("nb2", (128, MC), F32)
    inp("mb1", (128, MC), F32)
    inp("mb2", (128, ECH), F32)
    inp("gb2", (1, 1), F32)
    t["outp"] = nc.dram_tensor("outp", [R, E], F32, kind="ExternalOutput").ap()
    return t


def _emit(nc, tc, T):
    P = 128

    # ----- whole-kernel pools (left side) -----
    const = tc.alloc_tile_pool(name="const", bufs=1, side="left")
    wpool = tc.alloc_tile_pool(name="wts", bufs=2, side="left")
    misc = tc.alloc_tile_pool(name="misc", bufs=1, side="left")

    identb = const.tile([P, P], BF16, name="identb", tag="identb")
    make_identity(nc, identb)
    identf = const.tile([P, P], F32, name="identf", tag="identf")
    make_identity(nc, identf)
    ones_b = const.tile([P, 1], BF16, name="ones_b", tag="ones_b")
    nc.vector.memset(ones_b[:], 1.0)
    five_t = const.tile([1, 1], F32, name="five_t", tag="five_t")
    nc.vector.memset(five_t[:], 5.0)
    L36t = const.tile([P, K], BF16, name="L36t", tag="L36t")
    nc.sync.dma_start(L36t[:], T["L36"][:])
    selt = const.tile([4, 2], BF16, name="selt", tag="selt")
    nc.sync.dma_start(selt[:], T["sel"][:])
    Sbt = const.tile([P, 108], BF16, name="Sbt", tag="Sbt")
    nc.sync.dma_start(Sbt[:], T["Sb"][:])
    bmaskt = const.tile([108, 108], BF16, name="bmaskt", tag="bmaskt")
    nc.sync.dma_start(bmaskt[:], T["bmask"][:])
    rTt = const.tile([P, R], BF16, name="rTt", tag="rTt")
    nc.sync.dma_start(rTt[:], T["rT"][:])
    gw2t = const.tile([P, MC], FP8, name="gw2t", tag="gw2t")
    nc.sync.dma_start(gw2t[:], T["gw2"][:])
    btiles = {}
    for bn, cols in (("gb1", MC), ("nb1", MC), ("nb2", MC), ("mb1", MC), ("mb2", ECH)):
        bt = const.tile([P, cols], F32, name=bn + "t", tag=bn + "t")
        nc.sync.dma_start(bt[:], T[bn][:])
        btiles[bn] = bt
    gb2t = const.tile([1, 1], F32, name="gb2t", tag="gb2t")
    nc.sync.dma_start(gb2t[:], T["gb2"][:])
    bbT = misc.tile([4, R], BF16, name="bbT", tag="bbT")
    nc.sync.dma_start(bbT[:], T["bbT"][:])

    # ----- xT build: one [128, 18, R] fp8 tile -----
    pd_pool = tc.alloc_tile_pool(name="pd", bufs=2, space="PSUM")
    xpool = tc.alloc_tile_pool(name="xT", bufs=1, side="left")
    xTall = xpool.tile([P, KC1, R], FP8, name="xTall", tag="xTall")
    nc.sync.dma_start(xTall[:, 0:KC, :], T["imT"][:])
    nc.vector.memset(xTall[:, KC:KC1, :], 0.0)
    nc.vector.tensor_scalar_mul(xTall[0:4, KC, :], bbT[0:4, :], 0.1)
    d1s = misc.tile([1, R], F32, name="d1s", tag="d1s")
    for n in range(NT):
        sl = slice(n * NW, (n + 1) * NW)
        pd1 = pd_pool.tile([1, NW], F32, name=f"pd1_{n}", tag="pd1")
        nc.tensor.matmul(pd1[:], selt[:, 0:1], bbT[:, sl], start=True, stop=True)
        pd2 = pd_pool.tile([1, NW], F32, name=f"pd2_{n}", tag="pd2")
        nc.tensor.matmul(pd2[:], selt[:, 1:2], bbT[:, sl], start=True, stop=True)
        nc.scalar.copy(d1s[0:1, sl], pd1[:])
        # area*0.1 = (d2 * 0.1) * d1
        nc.vector.scalar_tensor_tensor(xTall[0:1, KC1 - 1, sl], pd2[:], 0.1,
                                       d1s[0:1, sl], OP.mult, OP.mult)
    pd_pool.release()

    # ----- MLP layer helpers (feature-major) -----
    DR = mybir.MatmulPerfMode.DoubleRow

    def mlp(lname, wap, nkc, bias_tile, func, rhs_at, out_pool, mcs, pspool,
            out_dtype=BF16):
        """bf16 layer: rhs_at(kc) returns a [128, R] AP."""
        outs = []
        for mc in range(mcs):
            wt = wpool.tile([P, nkc, P], BF16, name=f"w_{lname}_{mc}", tag="wstream",
                            padded_shape=[P, KC1, P])
            nc.sync.dma_start(wt[:], wap[mc])
            ot = out_pool.tile([P, R], out_dtype, name=f"{lname}_{mc}",
                               tag=f"{lname}_{mc}")
            for n in range(NT):
                ps = pspool.tile([P, NW], F32, name=f"ps_{lname}_{mc}_{n}", tag="pmm")
                for kc in range(nkc):
                    nc.tensor.matmul(ps[:], wt[:, kc, :],
                                     rhs_at(kc)[:, n * NW:(n + 1) * NW],
                                     start=(kc == 0), stop=(kc == nkc - 1))
                nc.scalar.activation(ot[:, n * NW:(n + 1) * NW], ps[:], func,
                                     bias=bias_tile[:, mc:mc + 1])
            outs.append(ot)
        return outs

    def mlp8(lname, wap, nkc, bias_tile, func, rhs_all, mcs, pspool,
             out_all=None, out_pool=None, out_dtype=BF16):
        """fp8 DoubleRow layer: rhs_all is one [128, nkc, R] fp8 tile;
        weights are pre-scaled by WSCALE, descaled in the ACT eviction.
        Writes into big tile out_all [128, mcs, R] if given, else returns
        a list of [128, R] tiles from out_pool."""
        outs = []
        for mc in range(mcs):
            wt = wpool.tile([P, nkc, P], FP8, name=f"w_{lname}_{mc}", tag="wstream8",
                            padded_shape=[P, KC1, P])
            nc.sync.dma_start(wt[:], wap[mc])
            if out_all is None:
                ot = out_pool.tile([P, R], out_dtype, name=f"{lname}_{mc}",
                                   tag=f"{lname}_{mc}")
                outs.append(ot)
            for n in range(NT):
                nsl = slice(n * NW, (n + 1) * NW)
                ps = pspool.tile([P, NW], F32, name=f"ps_{lname}_{mc}_{n}", tag="pmm")
                for kc in range(0, nkc, 2):
                    nc.tensor.matmul(ps[:], wt[:, kc:kc + 2, :],
                                     rhs_all[:, kc:kc + 2, nsl],
                                     start=(kc == 0), stop=(kc == nkc - 2),
                                     perf_mode=DR)
                dst = out_all[:, mc, nsl] if out_all is not None else ot[:, nsl]
                nc.scalar.activation(dst, ps[:], func, scale=1.0 / WSCALE,
                                     bias=bias_tile[:, mc:mc + 1])
        return outs

    # ----- gate MLP -----
    pmm = tc.alloc_tile_pool(name="pmm", bufs=6, space="PSUM")
    psg_pool = tc.alloc_tile_pool(name="psg", bufs=2, space="PSUM")
    hg_pool = tc.alloc_tile_pool(name="hg", bufs=1, side="left")
    hgall = hg_pool.tile([P, MC, R], FP8, name="hgall", tag="hgall")
    mlp8("hg", T["gw1"], KC1, btiles["gb1"], AF.Relu, xTall, MC, pmm,
         out_all=hgall)

    m_row = misc.tile([32, R], F32, name="m_row", tag="m_row")
    nc.vector.memset(m_row[:], 0.0)
    for n in range(NT):
        psg = psg_pool.tile([1, NW], F32, name=f"psg{n}", tag="psg")
        for kc in range(MC):
            nc.tensor.matmul(psg[:], gw2t[:, kc:kc + 1],
                             hgall[:, kc, n * NW:(n + 1) * NW],
                             start=(kc == 0), stop=(kc == MC - 1))
        nc.scalar.activation(m_row[0:1, n * NW:(n + 1) * NW], psg[:], AF.Sigmoid,
                             scale=1.0 / WSCALE, bias=gb2t[0:1, 0:1])
    psg_pool.release()
    hg_pool.release()

    # ----- W0 build (cumsum mask) + w = relu(5 - c) -----
    psc_pool = tc.alloc_tile_pool(name="psc", bufs=1, space="PSUM")
    w0a = misc.tile([P, R], BF16, name="w0a", tag="w0a")
    nc.vector.memset(w0a[:], 0.0)
    w_row = misc.tile([32, R], F32, name="w_row", tag="w_row")
    nc.vector.memset(w_row[:], 0.0)
    for n in range(NT):
        sl = slice(n * NW, (n + 1) * NW)
        psc = psc_pool.tile([K, NW], F32, name=f"psc{n}", tag="psc")
        nc.tensor.matmul(psc[:], L36t[:], rTt[:, sl], start=True, stop=True)
        nc.vector.scalar_tensor_tensor(w0a[0:K, sl], psc[:], 5.0, rTt[0:K, sl],
                                       OP.is_le, OP.mult)
        pcc = psc_pool.tile([1, NW], F32, name=f"pcc{n}", tag="pcc")
        nc.tensor.matmul(pcc[:], ones_b[:], rTt[:, sl], start=True, stop=True)
        nc.scalar.activation(w_row[0:1, sl], pcc[:], AF.Relu,
                             bias=five_t[0:1, 0:1], scale=-1.0)
    psc_pool.release()

    # ----- node MLP -----
    hn_pool = tc.alloc_tile_pool(name="hn", bufs=1, side="right")
    hnall = hn_pool.tile([P, MC, R], FP8, name="hnall", tag="hnall")
    mlp8("hn", T["nw1"], KC1, btiles["nb1"], AF.Relu, xTall, MC, pmm,
         out_all=hnall)
    xpool.release()
    v_pool = tc.alloc_tile_pool(name="v", bufs=1, side="left")
    v = mlp8("v", T["nw2"], KC, btiles["nb2"], AF.Identity, hnall, MC, pmm,
             out_pool=v_pool)
    hn_pool.release()
    pmm.release()

    # ----- message passing, per image group -----
    imgsT_pool = tc.alloc_tile_pool(name="imgsT", bufs=1, side="right")
    gpool = tc.alloc_tile_pool(name="gp", bufs=2, side="right")
    ptrv_pool = tc.alloc_tile_pool(name="ptrv", bufs=3, space="PSUM")
    ptr2_pool = tc.alloc_tile_pool(name="ptr2", bufs=2, space="PSUM")
    pmsg_pool = tc.alloc_tile_pool(name="pmsg", bufs=2, space="PSUM")
    pg2_pool = tc.alloc_tile_pool(name="pg2", bufs=1, space="PSUM")
    imgsTall = imgsT_pool.tile([P, MC, R], BF16, name="imgsTall", tag="imgsTall")

    for g, (i0, ng) in enumerate(GROUPS):
        rows = ng * K
        c0 = i0 * K

        # per-row scalars: mT (sigmoid gate) and wT (self-fallback count)
        pmw1 = ptrv_pool.tile([108, 32], F32, name=f"pmw1_{g}", tag="ptrv")
        nc.tensor.transpose(pmw1[0:rows, :], m_row[:, c0:c0 + rows],
                            identf[0:32, 0:32])
        pmw2 = ptrv_pool.tile([108, 32], F32, name=f"pmw2_{g}", tag="ptrv")
        nc.tensor.transpose(pmw2[0:rows, :], w_row[:, c0:c0 + rows],
                            identf[0:32, 0:32])
        mwT = gpool.tile([108, 2], F32, name=f"mwT{g}", tag="mwT")
        nc.vector.tensor_copy(mwT[0:rows, 0:1], pmw1[0:rows, 0:1])
        nc.vector.tensor_copy(mwT[0:rows, 1:2], pmw2[0:rows, 0:1])

        # block-diagonal stationary operand gW = blockdiag(W0^T) + diag(wT)
        pg2 = pg2_pool.tile([108, 108], F32, name=f"pg2_{g}", tag="pg2")
        nc.tensor.matmul(pg2[0:rows, 0:rows], Sbt[:, 0:rows],
                         w0a[:, c0:c0 + rows], start=True, stop=True)
        gW = gpool.tile([108, 108], BF16, name=f"gW{g}", tag="gW")
        nc.vector.tensor_mul(gW[0:rows, 0:rows], pg2[0:rows, 0:rows],
                             bmaskt[0:rows, 0:rows])
        tmpd = gpool.tile([108, 108], BF16, name=f"tmpd{g}", tag="tmpd")
        nc.vector.tensor_scalar_mul(tmpd[0:rows, 0:rows], identb[0:rows, 0:rows],
                                    mwT[0:rows, 1:2])
        nc.vector.tensor_add(gW[0:rows, 0:rows], gW[0:rows, 0:rows],
                             tmpd[0:rows, 0:rows])

        # u = m * v, transposed to row-major [rows, D]; 4 transposes share a
        # psum bank, the m-scale rides the ACT eviction (per-partition scale)
        u = gpool.tile([108, D], BF16, name=f"u{g}", tag="u")
        for q in range(4):
            psv4 = ptrv_pool.tile([108, 4, P], BF16, name=f"psv{g}_{q}",
                                  tag="ptrv")
            for j in range(4):
                dc = q * 4 + j
                nc.tensor.transpose(psv4[0:rows, j, :], v[dc][:, c0:c0 + rows],
                                    identb[:])
            nc.scalar.activation(u[0:rows, q * 512:(q + 1) * 512],
                                 psv4[0:rows, :, :], AF.Identity,
                                 scale=mwT[0:rows, 0:1])

        # out = gW^T @ u  [rows, D]; ACT evicts, DVE fuses square+rowsum
        out_sb = gpool.tile([108, D], F32, name=f"outsb{g}", tag="outsb")
        sq = gpool.tile([108, 512], F32, name=f"sq{g}", tag="sq")
        nsq = gpool.tile([108, 8], F32, name=f"nsq{g}", tag="nsq")
        for q in range(4):
            sl = slice(q * 512, (q + 1) * 512)
            psm = pmsg_pool.tile([108, 512], F32, name=f"psm{g}_{q}", tag="pmsg")
            nc.tensor.matmul(psm[0:rows, :], gW[0:rows, 0:rows], u[0:rows, sl],
                             start=True, stop=True)
            nc.scalar.copy(out_sb[0:rows, sl], psm[0:rows, :])
            nc.vector.scalar_tensor_tensor(sq[0:rows, :], out_sb[0:rows, sl],
                                           1.0, out_sb[0:rows, sl],
                                           OP.mult, OP.mult,
                                           accum_out=nsq[0:rows, q:q + 1])
        nc.vector.tensor_reduce(nsq[0:rows, 6:7], nsq[0:rows, 0:4],
                                axis=mybir.AxisListType.X, op=OP.add)
        nc.scalar.sqrt(nsq[0:rows, 7:8], nsq[0:rows, 6:7])
        inv = gpool.tile([108, 2], F32, name=f"inv{g}", tag="inv")
        nc.vector.tensor_scalar_add(inv[0:rows, 0:1], nsq[0:rows, 7:8], 1e-8)
        nc.vector.reciprocal(inv[0:rows, 1:2], inv[0:rows, 0:1])

        img_ld = gpool.tile([108, D], BF16, name=f"img{g}", tag="img")
        nc.sync.dma_start(img_ld[0:rows, :], T["im"][c0:c0 + rows, :])
        imgsw = gpool.tile([108, D], BF16, name=f"imgsw{g}", tag="imgsw")
        for q in range(4):
            sl = slice(q * 512, (q + 1) * 512)
            nc.vector.scalar_tensor_tensor(imgsw[0:rows, sl], out_sb[0:rows, sl],
                                           inv[0:rows, 1:2], img_ld[0:rows, sl],
                                           OP.mult, OP.add)

        # transpose back to feature-major imgsT; 4 transposes per psum bank,
        # one batched DVE copy each
        for q in range(4):
            pst4 = ptr2_pool.tile([P, 4, 108], BF16, name=f"pst4_{g}_{q}",
                                  tag="ptr2")
            for j in range(4):
                dc = q * 4 + j
                nc.tensor.transpose(pst4[:, j, 0:rows],
                                    imgsw[0:rows, dc * P:(dc + 1) * P],
                                    identb[0:rows, 0:rows])
            nc.vector.tensor_copy(imgsTall[:, q * 4:(q + 1) * 4, c0:c0 + rows],
                                  pst4[:, :, 0:rows])

    pg2_pool.release()
    pmsg_pool.release()
    ptr2_pool.release()
    ptrv_pool.release()
    gpool.release()
    v_pool.release()

    # ----- map MLP -----
    pmm2 = tc.alloc_tile_pool(name="pmm2", bufs=6, space="PSUM")
    hm_pool = tc.alloc_tile_pool(name="hm", bufs=1, side="left")
    hm = mlp("hm", T["mw1"], KC, btiles["mb1"], AF.Relu,
             lambda kc: imgsTall[:, kc, :], hm_pool, MC, pmm2)
    imgsT_pool.release()
    emb_pool = tc.alloc_tile_pool(name="emb", bufs=1, side="right")
    embT = mlp("embT", T["mw2"], KC, btiles["mb2"], AF.Identity,
               lambda kc: hm[kc][:], emb_pool, ECH, pmm2, out_dtype=F32)
    hm_pool.release()
    pmm2.release()

    # ----- final l2norm in row-major space, write row-major output -----
    pet_pool = tc.alloc_tile_pool(name="pet", bufs=2, space="PSUM")
    fin_pool = tc.alloc_tile_pool(name="fin", bufs=2, side="left")
    for g, (i0, ng) in enumerate(GROUPS):
        rows = ng * K
        c0 = i0 * K
        embR = fin_pool.tile([108, E], F32, name=f"embR{g}", tag="embR")
        sqf = fin_pool.tile([108, 512], F32, name=f"sqf{g}", tag="sqf")
        nsqf = fin_pool.tile([108, 8], F32, name=f"nsqf{g}", tag="nsqf")
        for q in range(2):
            pet4 = pet_pool.tile([108, 4, P], F32, name=f"pet{g}_{q}", tag="pet")
            for j in range(4):
                ec = q * 4 + j
                nc.tensor.transpose(pet4[0:rows, j, :], embT[ec][:, c0:c0 + rows],
                                    identf[:])
            sl = slice(q * 512, (q + 1) * 512)
            nc.scalar.copy(embR[0:rows, sl], pet4[0:rows, :, :])
            nc.vector.scalar_tensor_tensor(sqf[0:rows, :], embR[0:rows, sl],
                                           1.0, embR[0:rows, sl],
                                           OP.mult, OP.mult,
                                           accum_out=nsqf[0:rows, q:q + 1])
        nc.vector.tensor_add(nsqf[0:rows, 2:3], nsqf[0:rows, 0:1],
                             nsqf[0:rows, 1:2])
        nc.scalar.sqrt(nsqf[0:rows, 3:4], nsqf[0:rows, 2:3])
        nc.vector.tensor_scalar_add(nsqf[0:rows, 4:5], nsqf[0:rows, 3:4], 1e-8)
        nc.vector.reciprocal(nsqf[0:rows, 5:6], nsqf[0:rows, 4:5])
        embO = fin_pool.tile([108, E], F32, name=f"embO{g}", tag="embO")
        nc.vector.tensor_scalar_mul(embO[0:rows, :], embR[0:rows, :],
                                    nsqf[0:rows, 5:6])
        nc.sync.dma_start(T["outp"][c0:c0 + rows, :], embO[0:rows, :])
    pet_pool.release()
    fin_pool.release()
    emb_pool.release()
    misc.release()
    wpool.release()
    const.release()


def build_program(loop=1):
    nc = bacc.Bacc("TRN2", target_bir_lowering=False, debug=False,
                   num_devices=NCORES)
    T = _declare(nc)
    with tile.TileContext(nc) as tc:
        for _ in range(loop):
            _emit(nc, tc, T)
    nc.compile()
    return nc


# ------------------------------------------------------------- host glue ---

def _packw_aligned(w, nkc, dtype=NPBF16, scale=1.0):
    """(Kdim, M) fp32 -> (M/128, 128, nkc, 128)."""
    m = w.shape[1]
    mc = m // 128
    wp = np.asarray(w, np.float32) * scale
    if dtype is NPFP8:
        wp = np.clip(wp, -240.0, 240.0)
    return np.ascontiguousarray(
        wp.reshape(nkc, 128, mc, 128).transpose(2, 1, 0, 3)
    ).astype(dtype)


def _packw_x(w, dtype=NPBF16, scale=1.0):
    """(2053, M) fp32 -> 18-chunk layout: img dims 0..2047, bbox dims in
    chunk 16 rows 0..3, area dim in chunk 17 row 0."""
    m = w.shape[1]
    wp = np.zeros((KC1 * 128, m), np.float32)
    wp[:2048] = w[:2048]
    wp[2048:2052] = w[2048:2052]     # chunk 16, rows 0..3
    wp[17 * 128] = w[2052]           # chunk 17, row 0
    return _packw_aligned(wp, KC1, dtype=dtype, scale=scale)


def _bias(b):
    return np.ascontiguousarray(np.asarray(b, np.float32).reshape(-1, 128).T)


def prepare_inputs(inputs):
    images = np.asarray(inputs["images"], np.float32)
    bboxes = np.asarray(inputs["bboxes"], np.float32)
    img_range = np.asarray(inputs["img_range"], np.float32)

    sel = np.array([[-1.0, 0.0], [0.0, -1.0], [1.0, 0.0], [0.0, 1.0]], np.float32)
    Sb = np.zeros((128, 108), np.float32)
    for j in range(108):
        Sb[j % K, j] = 1.0
    bmask = np.zeros((108, 108), np.float32)
    for blk in range(3):
        bmask[blk * K:(blk + 1) * K, blk * K:(blk + 1) * K] = 1.0
    L36 = np.vstack([np.triu(np.ones((K, K), np.float32)),
                     np.zeros((128 - K, K), np.float32)])

    shared = {
        "gw1": _packw_x(np.asarray(inputs["gate_w1"], np.float32),
                        dtype=NPFP8, scale=WSCALE),
        "nw1": _packw_x(np.asarray(inputs["node_w1"], np.float32),
                        dtype=NPFP8, scale=WSCALE),
        "nw2": _packw_aligned(np.asarray(inputs["node_w2"], np.float32), KC,
                              dtype=NPFP8, scale=WSCALE),
        "mw1": _packw_aligned(np.asarray(inputs["map_w1"], np.float32), KC),
        "mw2": _packw_aligned(np.asarray(inputs["map_w2"], np.float32), KC),
        "gw2": np.ascontiguousarray(
            WSCALE * np.asarray(inputs["gate_w2"], np.float32).reshape(MC, 128).T
        ).astype(NPFP8),
        "gb1": _bias(inputs["gate_b1"]),
        "nb1": _bias(inputs["node_b1"]),
        "nb2": _bias(inputs["node_b2"]),
        "mb1": _bias(inputs["map_b1"]),
        "mb2": _bias(inputs["map_b2"]),
        "gb2": np.asarray(inputs["gate_b2"], np.float32).reshape(1, 1),
        "L36": L36.astype(NPBF16),
        "sel": sel.astype(NPBF16),
        "Sb": Sb.astype(NPBF16),
        "bmask": bmask.astype(NPBF16),
    }

    in_maps = []
    for c in range(NCORES):
        sl = slice(c * BSH, (c + 1) * BSH)
        imf = images[sl].reshape(R, D)
        rt = np.zeros((128, R), np.float32)
        rt[:K] = img_range[sl].transpose(2, 0, 1).reshape(K, R)
        m = dict(shared)
        m["imT"] = np.ascontiguousarray(
            imf.T.reshape(KC, 128, R).transpose(1, 0, 2)).astype(NPFP8)
        m["im"] = imf.astype(NPBF16)
        m["bbT"] = np.ascontiguousarray(bboxes[sl].reshape(R, 4).T).astype(NPBF16)
        m["rT"] = rt.astype(NPBF16)
        in_maps.append(m)
    return in_maps


def run(inputs, trace=False):
    nc = build_program()
    in_maps = prepare_inputs(inputs)
    res = run_bass_kernel_spmd(nc, in_maps, list(range(NCORES)), trace=trace)
    out = np.empty((B, K, E), np.float32)
    for c in range(NCORES):
        out[c * BSH:(c + 1) * BSH] = res.results[c]["outp"].reshape(BSH, K, E)
    return out, res


def kernel(**inputs):
    out, _ = run(inputs, trace=False)
    return out



# revision 31
# speedup vs baseline: 9.8930x; 1.0235x over previous
"""Trainium2 Bass kernel for nn_EncoderImage (gnn_message_passing).

Strategy: pure data-parallel over batch (32 images/core x 8 cores).
All MLP math runs feature-major ([feature_partitions, row_free]) so weight
tiles DMA directly as matmul lhsT and biases are per-partition ACT operands.

The topk/gather message passing is rewritten exactly (img_range values are
only 0/1, and jax.lax.top_k tie-breaks by lowest index):
    out[b] = (W0[b] + diag(relu(5 - c[b]))) @ (m[b] * v[b])
where W0[b][k,j] = r[k,j] * (cumsum_j r[k,:] <= 5) and c[b][k] = sum_j r[k,j].
The cumsum is a matmul against a constant triangular matrix; per 3-image
group the 36x36 W' blocks are assembled block-diagonally by a selector
matmul + block mask, so the whole group is one K=108 stationary operand.

Matmuls are bf16 (inputs rounded; fp32 PSUM accumulation); norms/biases fp32.

Hardware constraints honored: engine APs start at partition 0/32/64/96;
at most one PSUM input per DVE op; no tensor_tensor_reduce.
"""

import numpy as np
import ml_dtypes

import concourse.bacc as bacc
import concourse.bass as bass
import concourse.tile as tile
import concourse.mybir as mybir
from concourse.bass_utils import run_bass_kernel_spmd
from concourse.masks import make_identity

F32 = mybir.dt.float32
BF16 = mybir.dt.bfloat16
FP8 = mybir.dt.float8e4
AF = mybir.ActivationFunctionType
OP = mybir.AluOpType
NPBF16 = ml_dtypes.bfloat16
NPFP8 = ml_dtypes.float8_e4m3
WSCALE = 16.0

B, K, D, E = 256, 36, 2048, 1024
NCORES = 8
BSH = B // NCORES           # 32 images per core
R = BSH * K                 # 1152 rows per core
NT, NW = 3, 384             # row windows per psum accumulation group
KC1 = 18                    # k-chunks for x (2048 img + 4 bbox + 1 area)
KC = 16                     # k-chunks for D
MC = 16                     # m-chunks for D outputs
ECH = 8                     # m-chunks for E outputs
GROUPS = [(i, 3) for i in range(0, 30, 3)] + [(30, 2)]   # (start_img, n_imgs)


# ---------------------------------------------------------------- program ---

def _declare(nc):
    t = {}
    def inp(name, shape, dt):
        t[name] = nc.dram_tensor(name, list(shape), dt, kind="ExternalInput").ap()
    inp("imT", (128, KC, R), FP8)
    inp("im", (R, D), BF16)
    inp("bbT", (4, R), BF16)
    inp("rT", (128, R), BF16)
    inp("L36", (128, K), BF16)
    inp("sel", (4, 2), BF16)
    inp("Sb", (128, 108), BF16)
    inp("bmask", (108, 108), BF16)
    inp("gw1", (MC, 128, KC1, 128), FP8)
    inp("nw1", (MC, 128, KC1, 128), FP8)
    inp("nw2m", (128, KC, 2048), FP8)
    inp("nb2r", (1, 2048), BF16)
    inp("mw1", (MC, 128, KC, 128), BF16)
    inp("mw2", (ECH, 128, KC, 128), BF16)
    inp("gw2", (128, MC), FP8)
    inp("gb1", (128, MC), F32)
    inp("nb1", (128, MC), F32)
    inp("nb2", (128, MC), F32)
    inp("mb1", (128, MC), F32)
    inp("mb2", (128, ECH), F32)
    inp("gb2", (1, 1), F32)
    t["outp"] = nc.dram_tensor("outp", [R, E], F32, kind="ExternalOutput").ap()
    return t


def _emit(nc, tc, T):
    P = 128

    # ----- whole-kernel pools (left side) -----
    const = tc.alloc_tile_pool(name="const", bufs=1, side="left")
    wpool = tc.alloc_tile_pool(name="wts", bufs=2, side="left")
    misc = tc.alloc_tile_pool(name="misc", bufs=1, side="left")

    identb = const.tile([P, P], BF16, name="identb", tag="identb")
    make_identity(nc, identb)
    identf = const.tile([P, P], F32, name="identf", tag="identf")
    make_identity(nc, identf)
    ones_b = const.tile([P, 1], BF16, name="ones_b", tag="ones_b")
    nc.vector.memset(ones_b[:], 1.0)
    ones_r = const.tile([1, P], BF16, name="ones_r", tag="ones_r")
    nc.vector.memset(ones_r[:], 1.0)
    nb2r_t = const.tile([1, D], BF16, name="nb2r_t", tag="nb2r_t")
    nc.sync.dma_start(nb2r_t[:], T["nb2r"][:])
    five_t = const.tile([1, 1], F32, name="five_t", tag="five_t")
    nc.vector.memset(five_t[:], 5.0)
    L36t = const.tile([P, K], BF16, name="L36t", tag="L36t")
    nc.sync.dma_start(L36t[:], T["L36"][:])
    selt = const.tile([4, 2], BF16, name="selt", tag="selt")
    nc.sync.dma_start(selt[:], T["sel"][:])
    Sbt = const.tile([P, 108], BF16, name="Sbt", tag="Sbt")
    nc.sync.dma_start(Sbt[:], T["Sb"][:])
    bmaskt = const.tile([108, 108], BF16, name="bmaskt", tag="bmaskt")
    nc.sync.dma_start(bmaskt[:], T["bmask"][:])
    rTt = const.tile([P, R], BF16, name="rTt", tag="rTt")
    nc.sync.dma_start(rTt[:], T["rT"][:])
    gw2t = const.tile([P, MC], FP8, name="gw2t", tag="gw2t")
    nc.sync.dma_start(gw2t[:], T["gw2"][:])
    btiles = {}
    for bn, cols in (("gb1", MC), ("nb1", MC), ("nb2", MC), ("mb1", MC), ("mb2", ECH)):
        bt = const.tile([P, cols], F32, name=bn + "t", tag=bn + "t")
        nc.sync.dma_start(bt[:], T[bn][:])
        btiles[bn] = bt
    gb2t = const.tile([1, 1], F32, name="gb2t", tag="gb2t")
    nc.sync.dma_start(gb2t[:], T["gb2"][:])
    bbT = misc.tile([4, R], BF16, name="bbT", tag="bbT")
    nc.sync.dma_start(bbT[:], T["bbT"][:])

    # ----- xT build: one [128, 18, R] fp8 tile -----
    pd_pool = tc.alloc_tile_pool(name="pd", bufs=2, space="PSUM")
    nwm_pool = tc.alloc_tile_pool(name="nwm", bufs=1, side="left")
    nw2m = nwm_pool.tile([P, KC, D], FP8, name="nw2m", tag="nw2m")
    xpool = tc.alloc_tile_pool(name="xT", bufs=1, side="left")
    xTall = xpool.tile([P, KC1, R], FP8, name="xTall", tag="xTall")
    nc.sync.dma_start(xTall[:, 0:KC, :], T["imT"][:])
    nc.vector.memset(xTall[:, KC:KC1, :], 0.0)
    nc.vector.tensor_scalar_mul(xTall[0:4, KC, :], bbT[0:4, :], 0.1)
    d1s = misc.tile([1, R], F32, name="d1s", tag="d1s")
    for n in range(NT):
        sl = slice(n * NW, (n + 1) * NW)
        pd1 = pd_pool.tile([1, NW], F32, name=f"pd1_{n}", tag="pd1")
        nc.tensor.matmul(pd1[:], selt[:, 0:1], bbT[:, sl], start=True, stop=True)
        pd2 = pd_pool.tile([1, NW], F32, name=f"pd2_{n}", tag="pd2")
        nc.tensor.matmul(pd2[:], selt[:, 1:2], bbT[:, sl], start=True, stop=True)
        nc.scalar.copy(d1s[0:1, sl], pd1[:])
        # area*0.1 = (d2 * 0.1) * d1
        nc.vector.scalar_tensor_tensor(xTall[0:1, KC1 - 1, sl], pd2[:], 0.1,
                                       d1s[0:1, sl], OP.mult, OP.mult)
    pd_pool.release()

    # ----- MLP layer helpers (feature-major) -----
    DR = mybir.MatmulPerfMode.DoubleRow

    def mlp(lname, wap, nkc, bias_tile, func, rhs_at, out_pool, mcs, pspool,
            out_dtype=BF16):
        """bf16 layer: rhs_at(kc) returns a [128, R] AP."""
        outs = []
        for mc in range(mcs):
            wt = wpool.tile([P, nkc, P], BF16, name=f"w_{lname}_{mc}", tag="wstream",
                            padded_shape=[P, KC1, P])
            nc.sync.dma_start(wt[:], wap[mc])
            ot = out_pool.tile([P, R], out_dtype, name=f"{lname}_{mc}",
                               tag=f"{lname}_{mc}")
            for n in range(NT):
                ps = pspool.tile([P, NW], F32, name=f"ps_{lname}_{mc}_{n}", tag="pmm")
                for kc in range(nkc):
                    nc.tensor.matmul(ps[:], wt[:, kc, :],
                                     rhs_at(kc)[:, n * NW:(n + 1) * NW],
                                     start=(kc == 0), stop=(kc == nkc - 1))
                nc.scalar.activation(ot[:, n * NW:(n + 1) * NW], ps[:], func,
                                     bias=bias_tile[:, mc:mc + 1])
            outs.append(ot)
        return outs

    def mlp8(lname, wap, nkc, bias_tile, func, rhs_all, mcs, pspool,
             out_all=None, out_pool=None, out_dtype=BF16):
        """fp8 DoubleRow layer: rhs_all is one [128, nkc, R] fp8 tile;
        weights are pre-scaled by WSCALE, descaled in the ACT eviction.
        Writes into big tile out_all [128, mcs, R] if given, else returns
        a list of [128, R] tiles from out_pool."""
        outs = []
        for mc in range(mcs):
            wt = wpool.tile([P, nkc, P], FP8, name=f"w_{lname}_{mc}", tag="wstream8",
                            padded_shape=[P, KC1, P])
            nc.sync.dma_start(wt[:], wap[mc])
            if out_all is None:
                ot = out_pool.tile([P, R], out_dtype, name=f"{lname}_{mc}",
                                   tag=f"{lname}_{mc}")
                outs.append(ot)
            for n in range(NT):
                nsl = slice(n * NW, (n + 1) * NW)
                ps = pspool.tile([P, NW], F32, name=f"ps_{lname}_{mc}_{n}", tag="pmm")
                for kc in range(0, nkc, 2):
                    nc.tensor.matmul(ps[:], wt[:, kc:kc + 2, :],
                                     rhs_all[:, kc:kc + 2, nsl],
                                     start=(kc == 0), stop=(kc == nkc - 2),
                                     perf_mode=DR)
                dst = out_all[:, mc, nsl] if out_all is not None else ot[:, nsl]
                nc.scalar.activation(dst, ps[:], func, scale=1.0 / WSCALE,
                                     bias=bias_tile[:, mc:mc + 1])
        return outs

    # ----- gate MLP -----
    pmm = tc.alloc_tile_pool(name="pmm", bufs=6, space="PSUM")
    psg_pool = tc.alloc_tile_pool(name="psg", bufs=2, space="PSUM")
    hg_pool = tc.alloc_tile_pool(name="hg", bufs=1, side="left")
    hgall = hg_pool.tile([P, MC, R], FP8, name="hgall", tag="hgall")
    mlp8("hg", T["gw1"], KC1, btiles["gb1"], AF.Relu, xTall, MC, pmm,
         out_all=hgall)

    m_row = misc.tile([32, R], F32, name="m_row", tag="m_row")
    nc.vector.memset(m_row[:], 0.0)
    for n in range(NT):
        psg = psg_pool.tile([1, NW], F32, name=f"psg{n}", tag="psg")
        for kc in range(MC):
            nc.tensor.matmul(psg[:], gw2t[:, kc:kc + 1],
                             hgall[:, kc, n * NW:(n + 1) * NW],
                             start=(kc == 0), stop=(kc == MC - 1))
        nc.scalar.activation(m_row[0:1, n * NW:(n + 1) * NW], psg[:], AF.Sigmoid,
                             scale=1.0 / WSCALE, bias=gb2t[0:1, 0:1])
    psg_pool.release()
    hg_pool.release()

    # ----- W0 build (cumsum mask) + w = relu(5 - c) -----
    psc_pool = tc.alloc_tile_pool(name="psc", bufs=1, space="PSUM")
    w0a = misc.tile([P, R], BF16, name="w0a", tag="w0a")
    nc.vector.memset(w0a[:], 0.0)
    w_row = misc.tile([32, R], F32, name="w_row", tag="w_row")
    nc.vector.memset(w_row[:], 0.0)
    for n in range(NT):
        sl = slice(n * NW, (n + 1) * NW)
        psc = psc_pool.tile([K, NW], F32, name=f"psc{n}", tag="psc")
        nc.tensor.matmul(psc[:], L36t[:], rTt[:, sl], start=True, stop=True)
        nc.vector.scalar_tensor_tensor(w0a[0:K, sl], psc[:], 5.0, rTt[0:K, sl],
                                       OP.is_le, OP.mult)
        pcc = psc_pool.tile([1, NW], F32, name=f"pcc{n}", tag="pcc")
        nc.tensor.matmul(pcc[:], ones_b[:], rTt[:, sl], start=True, stop=True)
        nc.scalar.activation(w_row[0:1, sl], pcc[:], AF.Relu,
                             bias=five_t[0:1, 0:1], scale=-1.0)
    psc_pool.release()

    # ----- node MLP layer 1; layer 2 runs row-major inside the group loop -----
    hn_pool = tc.alloc_tile_pool(name="hn", bufs=1, side="right")
    hnall = hn_pool.tile([P, MC, R], FP8, name="hnall", tag="hnall")
    mlp8("hn", T["nw1"], KC1, btiles["nb1"], AF.Relu, xTall, MC, pmm,
         out_all=hnall)
    xpool.release()
    pmm.release()
    nc.sync.dma_start(nw2m[:], T["nw2m"][:])

    # ----- message passing, per image group -----
    imgsT_pool = tc.alloc_tile_pool(name="imgsT", bufs=1, side="right")
    gpool = tc.alloc_tile_pool(name="gp", bufs=3, side="right")
    ptrv_pool = tc.alloc_tile_pool(name="ptrv", bufs=3, space="PSUM")
    ptr2_pool = tc.alloc_tile_pool(name="ptr2", bufs=3, space="PSUM")
    pmsg_pool = tc.alloc_tile_pool(name="pmsg", bufs=3, space="PSUM")
    pg2_pool = tc.alloc_tile_pool(name="pg2", bufs=1, space="PSUM")
    imgsTall = imgsT_pool.tile([P, MC, R], BF16, name="imgsTall", tag="imgsTall")

    for g, (i0, ng) in enumerate(GROUPS):
        rows = ng * K
        c0 = i0 * K

        # per-row scalars: mT (sigmoid gate) and wT (self-fallback count)
        pmw1 = ptrv_pool.tile([108, 32], F32, name=f"pmw1_{g}", tag="ptrv")
        nc.tensor.transpose(pmw1[0:rows, :], m_row[:, c0:c0 + rows],
                            identf[0:32, 0:32])
        pmw2 = ptrv_pool.tile([108, 32], F32, name=f"pmw2_{g}", tag="ptrv")
        nc.tensor.transpose(pmw2[0:rows, :], w_row[:, c0:c0 + rows],
                            identf[0:32, 0:32])
        mwT = gpool.tile([108, 2], F32, name=f"mwT{g}", tag="mwT")
        nc.vector.tensor_scalar_mul(mwT[0:rows, 0:1], pmw1[0:rows, 0:1],
                                    1.0 / WSCALE)
        nc.vector.tensor_copy(mwT[0:rows, 1:2], pmw2[0:rows, 0:1])

        # block-diagonal stationary operand gW = blockdiag(W0^T) + diag(wT)
        pg2 = pg2_pool.tile([108, 108], F32, name=f"pg2_{g}", tag="pg2")
        nc.tensor.matmul(pg2[0:rows, 0:rows], Sbt[:, 0:rows],
                         w0a[:, c0:c0 + rows], start=True, stop=True)
        gW = gpool.tile([108, 108], BF16, name=f"gW{g}", tag="gW")
        nc.vector.tensor_mul(gW[0:rows, 0:rows], pg2[0:rows, 0:rows],
                             bmaskt[0:rows, 0:rows])
        tmpd = gpool.tile([108, 108], BF16, name=f"tmpd{g}", tag="tmpd")
        nc.vector.tensor_scalar_mul(tmpd[0:rows, 0:rows], identb[0:rows, 0:rows],
                                    mwT[0:rows, 1:2])
        nc.vector.tensor_add(gW[0:rows, 0:rows], gW[0:rows, 0:rows],
                             tmpd[0:rows, 0:rows])

        # u = m * v, built row-major directly: node layer 2 with stationary
        # hnall slices and moving nw2m; m/WSCALE rides the ACT eviction
        u = gpool.tile([108, D], BF16, name=f"u{g}", tag="u")
        for q in range(4):
            sl = slice(q * 512, (q + 1) * 512)
            pv = ptrv_pool.tile([108, 512], F32, name=f"pv{g}_{q}", tag="ptrv")
            nc.tensor.matmul(pv[0:rows, :], ones_r[0:1, 0:rows],
                             nb2r_t[0:1, sl], start=True, stop=False)
            for kc in range(0, KC, 2):
                nc.tensor.matmul(pv[0:rows, :],
                                 hnall[:, kc:kc + 2, c0:c0 + rows],
                                 nw2m[:, kc:kc + 2, sl],
                                 start=False, stop=(kc == KC - 2),
                                 perf_mode=DR)
            nc.scalar.activation(u[0:rows, sl], pv[0:rows, :], AF.Identity,
                                 scale=mwT[0:rows, 0:1])

        # out = gW^T @ u  [rows, D]; ACT evicts, DVE fuses square+rowsum
        out_sb = gpool.tile([108, D], F32, name=f"outsb{g}", tag="outsb")
        sq = gpool.tile([108, 512], F32, name=f"sq{g}", tag="sq")
        nsq = gpool.tile([108, 8], F32, name=f"nsq{g}", tag="nsq")
        for q in range(4):
            sl = slice(q * 512, (q + 1) * 512)
            psm = pmsg_pool.tile([108, 512], F32, name=f"psm{g}_{q}", tag="pmsg")
            nc.tensor.matmul(psm[0:rows, :], gW[0:rows, 0:rows], u[0:rows, sl],
                             start=True, stop=True)
            nc.scalar.copy(out_sb[0:rows, sl], psm[0:rows, :])
            nc.vector.scalar_tensor_tensor(sq[0:rows, :], out_sb[0:rows, sl],
                                           1.0, out_sb[0:rows, sl],
                                           OP.mult, OP.mult,
                                           accum_out=nsq[0:rows, q:q + 1])
        nc.vector.tensor_reduce(nsq[0:rows, 6:7], nsq[0:rows, 0:4],
                                axis=mybir.AxisListType.X, op=OP.add)
        nc.scalar.sqrt(nsq[0:rows, 7:8], nsq[0:rows, 6:7])
        inv = gpool.tile([108, 2], F32, name=f"inv{g}", tag="inv")
        nc.vector.tensor_scalar_add(inv[0:rows, 0:1], nsq[0:rows, 7:8], 1e-8)
        nc.vector.reciprocal(inv[0:rows, 1:2], inv[0:rows, 0:1])

        img_ld = gpool.tile([108, D], BF16, name=f"img{g}", tag="img")
        nc.sync.dma_start(img_ld[0:rows, :], T["im"][c0:c0 + rows, :])
        imgsw = gpool.tile([108, D], BF16, name=f"imgsw{g}", tag="imgsw")
        for q in range(4):
            sl = slice(q * 512, (q + 1) * 512)
            nc.vector.scalar_tensor_tensor(imgsw[0:rows, sl], out_sb[0:rows, sl],
                                           inv[0:rows, 1:2], img_ld[0:rows, sl],
                                           OP.mult, OP.add)

        # transpose back to feature-major imgsT; 4 transposes per psum bank,
        # one batched DVE copy each
        for q in range(4):
            pst4 = ptr2_pool.tile([P, 4, 108], BF16, name=f"pst4_{g}_{q}",
                                  tag="ptr2")
            for j in range(4):
                dc = q * 4 + j
                nc.tensor.transpose(pst4[:, j, 0:rows],
                                    imgsw[0:rows, dc * P:(dc + 1) * P],
                                    identb[0:rows, 0:rows])
            nc.vector.tensor_copy(imgsTall[:, q * 4:(q + 1) * 4, c0:c0 + rows],
                                  pst4[:, :, 0:rows])

    pg2_pool.release()
    pmsg_pool.release()
    ptr2_pool.release()
    ptrv_pool.release()
    gpool.release()
    nwm_pool.release()

    # ----- map MLP -----
    pmm2 = tc.alloc_tile_pool(name="pmm2", bufs=6, space="PSUM")
    hm_pool = tc.alloc_tile_pool(name="hm", bufs=1, side="left")
    hm = mlp("hm", T["mw1"], KC, btiles["mb1"], AF.Relu,
             lambda kc: imgsTall[:, kc, :], hm_pool, MC, pmm2)
    imgsT_pool.release()
    hn_pool.release()

    # ----- map layer 2 window-major, final l2norm groups interleaved so the
    # tail overlaps remaining embT windows -----
    emb_pool = tc.alloc_tile_pool(name="emb", bufs=1, side="right")
    w8pool = tc.alloc_tile_pool(name="w8", bufs=8, side="right")
    pet_pool = tc.alloc_tile_pool(name="pet", bufs=2, space="PSUM")
    fin_pool = tc.alloc_tile_pool(name="fin", bufs=2, side="left")
    embT = [emb_pool.tile([P, R], F32, name=f"embT_{ec}", tag=f"embT_{ec}")
            for ec in range(ECH)]
    wts2 = []
    for ec in range(ECH):
        wt = w8pool.tile([P, KC, P], BF16, name=f"w_embT_{ec}", tag="w2s")
        nc.sync.dma_start(wt[:], T["mw2"][ec])
        wts2.append(wt)

    def fin_group(g):
        i0, ng = GROUPS[g]
        rows = ng * K
        c0 = i0 * K
        embR = fin_pool.tile([108, E], F32, name=f"embR{g}", tag="embR")
        sqf = fin_pool.tile([108, 512], F32, name=f"sqf{g}", tag="sqf")
        nsqf = fin_pool.tile([108, 8], F32, name=f"nsqf{g}", tag="nsqf")
        for q in range(2):
            pet4 = pet_pool.tile([108, 4, P], F32, name=f"pet{g}_{q}", tag="pet")
            for j in range(4):
                ec = q * 4 + j
                nc.tensor.transpose(pet4[0:rows, j, :], embT[ec][:, c0:c0 + rows],
                                    identf[:])
            sl = slice(q * 512, (q + 1) * 512)
            nc.scalar.copy(embR[0:rows, sl], pet4[0:rows, :, :])
            nc.vector.scalar_tensor_tensor(sqf[0:rows, :], embR[0:rows, sl],
                                           1.0, embR[0:rows, sl],
                                           OP.mult, OP.mult,
                                           accum_out=nsqf[0:rows, q:q + 1])
        nc.vector.tensor_add(nsqf[0:rows, 2:3], nsqf[0:rows, 0:1],
                             nsqf[0:rows, 1:2])
        nc.scalar.sqrt(nsqf[0:rows, 3:4], nsqf[0:rows, 2:3])
        nc.vector.tensor_scalar_add(nsqf[0:rows, 4:5], nsqf[0:rows, 3:4], 1e-8)
        nc.vector.reciprocal(nsqf[0:rows, 5:6], nsqf[0:rows, 4:5])
        embO = fin_pool.tile([108, E], F32, name=f"embO{g}", tag="embO")
        nc.vector.tensor_scalar_mul(embO[0:rows, :], embR[0:rows, :],
                                    nsqf[0:rows, 5:6])
        nc.sync.dma_start(T["outp"][c0:c0 + rows, :], embO[0:rows, :])

    win_groups = {0: range(0, 3), 1: range(3, 7), 2: range(7, 11)}
    for n in range(NT):
        nsl = slice(n * NW, (n + 1) * NW)
        for ec in range(ECH):
            ps = pmm2.tile([P, NW], F32, name=f"ps_embT_{ec}_{n}", tag="pmm")
            for kc in range(KC):
                nc.tensor.matmul(ps[:], wts2[ec][:, kc, :], hm[kc][:, nsl],
                                 start=(kc == 0), stop=(kc == KC - 1))
            nc.scalar.activation(embT[ec][:, nsl], ps[:], AF.Identity,
                                 bias=btiles["mb2"][:, ec:ec + 1])
        for g in win_groups[n]:
            fin_group(g)
    fin_pool.release()
    hm_pool.release()
    pet_pool.release()
    pmm2.release()
    w8pool.release()
    emb_pool.release()
    misc.release()
    wpool.release()
    const.release()


def build_program(loop=1):
    nc = bacc.Bacc("TRN2", target_bir_lowering=False, debug=False,
                   num_devices=NCORES)
    T = _declare(nc)
    with tile.TileContext(nc) as tc:
        for _ in range(loop):
            _emit(nc, tc, T)
    nc.compile()
    return nc


# ------------------------------------------------------------- host glue ---

def _packw_aligned(w, nkc, dtype=NPBF16, scale=1.0):
    """(Kdim, M) fp32 -> (M/128, 128, nkc, 128)."""
    m = w.shape[1]
    mc = m // 128
    wp = np.asarray(w, np.float32) * scale
    if dtype is NPFP8:
        wp = np.clip(wp, -240.0, 240.0)
    return np.ascontiguousarray(
        wp.reshape(nkc, 128, mc, 128).transpose(2, 1, 0, 3)
    ).astype(dtype)


def _packw_x(w, dtype=NPBF16, scale=1.0):
    """(2053, M) fp32 -> 18-chunk layout: img dims 0..2047, bbox dims in
    chunk 16 rows 0..3, area dim in chunk 17 row 0."""
    m = w.shape[1]
    wp = np.zeros((KC1 * 128, m), np.float32)
    wp[:2048] = w[:2048]
    wp[2048:2052] = w[2048:2052]     # chunk 16, rows 0..3
    wp[17 * 128] = w[2052]           # chunk 17, row 0
    return _packw_aligned(wp, KC1, dtype=dtype, scale=scale)


def _bias(b):
    return np.ascontiguousarray(np.asarray(b, np.float32).reshape(-1, 128).T)


def prepare_inputs(inputs):
    images = np.asarray(inputs["images"], np.float32)
    bboxes = np.asarray(inputs["bboxes"], np.float32)
    img_range = np.asarray(inputs["img_range"], np.float32)

    sel = np.array([[-1.0, 0.0], [0.0, -1.0], [1.0, 0.0], [0.0, 1.0]], np.float32)
    Sb = np.zeros((128, 108), np.float32)
    for j in range(108):
        Sb[j % K, j] = 1.0
    bmask = np.zeros((108, 108), np.float32)
    for blk in range(3):
        bmask[blk * K:(blk + 1) * K, blk * K:(blk + 1) * K] = 1.0
    L36 = np.vstack([np.triu(np.ones((K, K), np.float32)),
                     np.zeros((128 - K, K), np.float32)])

    shared = {
        "gw1": _packw_x(np.asarray(inputs["gate_w1"], np.float32),
                        dtype=NPFP8, scale=WSCALE),
        "nw1": _packw_x(np.asarray(inputs["node_w1"], np.float32),
                        dtype=NPFP8, scale=WSCALE),
        "nw2m": np.ascontiguousarray(
            np.clip(np.asarray(inputs["node_w2"], np.float32) * WSCALE,
                    -240.0, 240.0).reshape(KC, 128, D).transpose(1, 0, 2)
        ).astype(NPFP8),
        "nb2r": (np.asarray(inputs["node_b2"], np.float32) * WSCALE
                 ).reshape(1, D).astype(NPBF16),
        "mw1": _packw_aligned(np.asarray(inputs["map_w1"], np.float32), KC),
        "mw2": _packw_aligned(np.asarray(inputs["map_w2"], np.float32), KC),
        "gw2": np.ascontiguousarray(
            WSCALE * np.asarray(inputs["gate_w2"], np.float32).reshape(MC, 128).T
        ).astype(NPFP8),
        "gb1": _bias(inputs["gate_b1"]),
        "nb1": _bias(inputs["node_b1"]),
        "nb2": _bias(inputs["node_b2"]),
        "mb1": _bias(inputs["map_b1"]),
        "mb2": _bias(inputs["map_b2"]),
        "gb2": np.asarray(inputs["gate_b2"], np.float32).reshape(1, 1),
        "L36": L36.astype(NPBF16),
        "sel": sel.astype(NPBF16),
        "Sb": Sb.astype(NPBF16),
        "bmask": bmask.astype(NPBF16),
    }

    in_maps = []
    for c in range(NCORES):
        sl = slice(c * BSH, (c + 1) * BSH)
        imf = images[sl].reshape(R, D)
        rt = np.zeros((128, R), np.float32)
        rt[:K] = img_range[sl].transpose(2, 0, 1).reshape(K, R)
        m = dict(shared)
        m["imT"] = np.ascontiguousarray(
            imf.T.reshape(KC, 128, R).transpose(1, 0, 2)).astype(NPFP8)
        m["im"] = imf.astype(NPBF16)
        m["bbT"] = np.ascontiguousarray(bboxes[sl].reshape(R, 4).T).astype(NPBF16)
        m["rT"] = rt.astype(NPBF16)
        in_maps.append(m)
    return in_maps


def run(inputs, trace=False):
    nc = build_program()
    in_maps = prepare_inputs(inputs)
    res = run_bass_kernel_spmd(nc, in_maps, list(range(NCORES)), trace=trace)
    out = np.empty((B, K, E), np.float32)
    for c in range(NCORES):
        out[c * BSH:(c + 1) * BSH] = res.results[c]["outp"].reshape(BSH, K, E)
    return out, res


def kernel(**inputs):
    out, _ = run(inputs, trace=False)
    return out

